# revision 23
# baseline (speedup 1.0000x reference)
"""Trainium2 Bass kernel for nn_Decoder_75892072120909 (sparse-attention decoder).

Self-contained: takes FULL inputs (as produced by the problem's setup_inputs),
runs an 8-core SPMD Bass kernel, returns the FULL output [2, 1920, 1024].

Sharding: 2 batches x 4 cores; each core owns 4 frame blocks (the last core of
a batch owns frames [11,12,13,14]; frame 11 is taken from the previous core so
every core runs the identical SPMD program). Each core also replicates the
tiny "non-frame" token trajectory (delim + dynamics tokens, 12 per block = 180
per batch) whose attention is the identity (those tokens attend only to
themselves), so no cross-core communication is needed.

On-core layout: activations are feature-on-partition ("transposed world",
hT [512, Ntok]) so every matmul consumes operands natively. The residual
stream hT stays fp32(r); everything downstream of the LayerNorms (LN outputs,
q/k/v, exp'd scores, gelu outputs and all weights) is bf16, which keeps every
matmul at 1 cycle/row including the N=128 frame-attention blocks (f32r drops
to 4 cycles/row below N=256). Softmax denominators use the single-instruction
DVE reciprocal_approx_fast (~18 bits) instead of the 8-cycle iterative
reciprocal. Non-frame tokens get no q (they are never attention queries) and
in the last layer skip o-proj/LN2/MLP (their logits are discarded).
LayerNorm statistics use ones-vector matmuls on the tensor engine;
rstd = exp(-0.5*ln(var+eps)) keeps the scalar engine on the natural_log_exp
table set shared with the attention exponentials. LN1 of layer l+1 is emitted
inside layer l's MLP tail so its stats/broadcast overlap the down-proj stream.
"""

import sys
import numpy as np

for _p in ("/opt/trn_rl_repo", "/root/.axon_site/_ro/trn_rl_repo"):
    if _p not in sys.path:
        sys.path.append(_p)

import ml_dtypes
import concourse.bass as bass
import concourse.tile as tile
from concourse import mybir
from concourse.bass_utils import run_bass_kernel_spmd

# ---------------- problem constants (hardcoded) ----------------
F = 128           # frame tokens per block
T = 10            # dynamics tokens per block
BLK = F + T + 2   # 140
N = 15            # frame blocks
B = 2
W = 512
L = 8
H = 8
DH = 64
S = N * BLK       # 2100
NNF = N * (T + 2)  # 180 non-frame tokens per batch
NQ = 4 * F        # 512 frame-token queries per core
NTOK = NNF + NQ   # 692 tokens per core
EPS = 1e-5
NEG = -1e30
SCALE = 1.0 / np.sqrt(DH)
CORE_FRAMES = [[0, 1, 2, 3], [4, 5, 6, 7], [8, 9, 10, 11], [11, 12, 13, 14]]
F32 = mybir.dt.float32
F32R = mybir.dt.float32r
BF16 = mybir.dt.bfloat16
AF = mybir.ActivationFunctionType
OP = mybir.AluOpType
TP = ((0, 346), (346, NTOK))        # full token passes
TPL = ((NNF, 436), (436, NTOK))     # last-layer passes (frame tokens only)
# attention-aligned token chunks: 0-1 = the 180 nf tokens, 2-5 = frame tokens
TCH = ((0, 128), (128, 180), (180, 308), (308, 436), (436, 564), (564, 692))


def build_program(waitsplit=True, ln_pb=True, att_pb=True, att_pipe=True, psum_rec=False):
    nc = bass.Bass("TRN2", target_bir_lowering=False, debug=False, num_devices=8)

    h0 = nc.dram_tensor("h0", [W, NTOK], F32, kind="ExternalInput").ap()
    wqkvT_d = nc.dram_tensor("wqkvT", [L, W, 3 * W], BF16, kind="ExternalInput").ap()
    woT_d = nc.dram_tensor("woT", [L, W, W], BF16, kind="ExternalInput").ap()
    wfcT_d = nc.dram_tensor("wfcT", [L, W, 4 * W], BF16, kind="ExternalInput").ap()
    wprojT_d = nc.dram_tensor("wprojT", [L, 4 * W, W], BF16, kind="ExternalInput").ap()
    predT_d = nc.dram_tensor("predT", [W, 1024], F32, kind="ExternalInput").ap()
    mask_d = nc.dram_tensor("nfmask", [NNF, 512], F32, kind="ExternalInput").ap()
    cvec_d = nc.dram_tensor("cvec", [128, 4], F32, kind="ExternalInput").ap()
    out_d = nc.dram_tensor("logits", [NQ, 1024], F32, kind="ExternalOutput").ap()

    with tile.TileContext(nc) as tc:
        _build(tc, h0, wqkvT_d, woT_d, wfcT_d, wprojT_d, predT_d, mask_d,
               cvec_d, out_d, ln_pb, att_pb, att_pipe, psum_rec)

    from concourse.library_overlay import lower_extended_insts
    lower_extended_insts(nc)
    if waitsplit:
        from waitsplit_embedded import split_excess_waits
        split_excess_waits(nc)
    return nc


def _build(tc, h0, wqkvT_d, woT_d, wfcT_d, wprojT_d, predT_d, mask_d,
           cvec_d, out_d, ln_pb=True, att_pb=True, att_pipe=True, psum_rec=True):
    nc = tc.nc
    from contextlib import ExitStack
    ctx = ExitStack()

    def pool(name, bufs, **kw):
        return ctx.enter_context(tc.tile_pool(name=name, bufs=bufs, **kw))

    state = pool("state", 1)
    apool = pool("apool", 3)
    qkp = pool("qkp", 1)
    vp = pool("vp", 1)
    attp = pool("attp", 1)
    ep = pool("ep", 5)
    eop = pool("eop", 3)
    up = pool("up", 3)
    sq = pool("sq", 4)
    sp = pool("sp", 2)
    rp = pool("rp", 2)
    bc = pool("bc", 2)
    wq_p = pool("wq", 1)
    wo_p = pool("wo", 1)
    wfc_p = pool("wfc", 1)
    wpr_p = pool("wpr", 1)
    cst = pool("cst", 1)
    lout = pool("lout", 2)

    # PSUM: 8 banks. psA(mm)x4 rotate through everything transient (LN stats,
    # qkv/v/score/o-proj/up psums); psM g0..g3 are the long-lived accumulators
    # (per-head attention oT during attention, MLP down-proj acc during the
    # MLP). All broadcasts go through gpsimd partition_broadcast into SBUF so
    # the tensor engine never sits behind a scalar/vector chain.
    psA = pool("psA", 4, space="PSUM")
    psM = pool("psM", 1, space="PSUM")

    # ---- constants ----
    ones_inv = cst.tile([128, 1], F32R, name="ones_inv")   # value 1/512
    nc.sync.dma_start(out=ones_inv, in_=cvec_d[:, 0:1].bitcast(F32R))
    cvec_t = cst.tile([128, 4], F32, name="cvec_t")
    nc.sync.dma_start(out=cvec_t, in_=cvec_d)
    ones_row = cst.tile([1, 128], F32R, name="ones_row")
    nc.sync.dma_start(out=ones_row, in_=cvec_d[:, 1:2].rearrange("p o -> o p").bitcast(F32R))
    maskb_t = cst.tile([128, 2, 512], F32, name="maskb_t")
    nc.sync.dma_start(out=maskb_t[0:128, 0, :], in_=mask_d[0:128, :])
    nc.sync.dma_start(out=maskb_t[0:52, 1, :], in_=mask_d[128:180, :])

    # ---- persistent activations ----
    hT = state.tile([128, 4, NTOK], F32R, name="hT")
    nc.sync.dma_start(out=hT, in_=h0.rearrange("(c p) t -> p c t", p=128).bitcast(F32R))
    qkT = qkp.tile([128, 8, NTOK], BF16, name="qkT")
    v_aug = vp.tile([128, 6, H, DH + 1], BF16, name="v_aug")
    for ci in range(6):
        nc.gpsimd.memset(v_aug[:, ci, :, DH:DH + 1], 1.0)
    attnT = attp.tile([128, 4, NTOK], BF16, name="attnT")

    # partition_broadcast lives in the Q7 "proxy" ucode library
    from concourse import library_config
    nc.gpsimd.load_library(library_config.proxy)

    def ln_stats(t0, t1, vt, row):
        """PE stats matmuls + vector mu/var chain for tokens [t0,t1).
        Writes the variance into row `row` of the shared tile `vt` so the
        scalar Ln/Exp of both passes run as ONE pair of instructions (keeps
        the act-table flips at exactly 2 per layer)."""
        n = t1 - t0
        mu_ps = psA.tile([1, 512], F32, tag="mm", name=f"mu{nc.next_id()}")
        ms_ps = psA.tile([1, 512], F32, tag="mm", name=f"ms{nc.next_id()}")
        for c in range(4):
            nc.tensor.matmul(mu_ps[:, 0:n], ones_inv, hT[:, c, t0:t1],
                             start=(c == 0), stop=(c == 3))
        for c in range(4):
            hsq = sq.tile([128, 346], F32R, tag="hsq", name=f"hsq{nc.next_id()}")
            nc.gpsimd.tensor_mul(hsq[:, 0:n], hT[:, c, t0:t1], hT[:, c, t0:t1])
            nc.tensor.matmul(ms_ps[:, 0:n], ones_inv, hsq[:, 0:n],
                             start=(c == 0), stop=(c == 3))
        mu = sp.tile([1, 346], F32R, tag="mu", name=f"muv{nc.next_id()}")
        nc.vector.tensor_copy(out=mu[:, 0:n], in_=mu_ps[:, 0:n])
        # broadcast of mu can start as soon as the copy lands
        mu_b = bc.tile([128, 346], F32R, tag="mu_b", name=f"mub{nc.next_id()}")
        nc.gpsimd.partition_broadcast(mu_b[:, 0:n], mu[:, 0:n])
        musq = sp.tile([1, 346], F32, tag="musq", name=f"mq{nc.next_id()}")
        nc.vector.tensor_mul(musq[:, 0:n], mu[:, 0:n], mu[:, 0:n])
        nc.vector.tensor_tensor(out=vt[0:1, row, 0:n], in0=ms_ps[:, 0:n],
                                in1=musq[:, 0:n], op=OP.subtract)
        return mu_b

    def ln_var_tile():
        # both passes' variances batched along the free dim of one partition
        return sp.tile([1, 2, 346], F32, tag="var", name=f"var{nc.next_id()}")

    def ln_finish(dst, mu_bs, vt, passes):
        """one batched Ln+Exp for both passes, then per-pass bcast + apply."""
        nn = passes[0][1] - passes[0][0]
        assert all(t1 - t0 == nn for t0, t1 in passes)
        lnv = sp.tile([1, 2, 346], F32, tag="lnv", name=f"lnv{nc.next_id()}")
        nc.scalar.activation(lnv[:, :, 0:nn], vt[:, :, 0:nn], AF.Ln,
                             bias=cvec_t[0:1, 2:3])
        rstd = sp.tile([1, 2, 346], F32R, tag="rstd", name=f"rsd{nc.next_id()}")
        nc.scalar.activation(rstd[:, :, 0:nn], lnv[:, :, 0:nn],
                             AF.Exp, scale=-0.5)
        for pi, (t0, t1) in enumerate(passes):
            n = t1 - t0
            rs_b = bc.tile([128, 346], F32R, tag="rs_b", name=f"rsb{nc.next_id()}")
            nc.gpsimd.partition_broadcast(rs_b[:, 0:n], rstd[0:1, pi, 0:n])
            mu_b = mu_bs[pi]
            for c in range(4):
                nc.vector.tensor_tensor(out=dst[:, c, t0:t1], in0=hT[:, c, t0:t1],
                                        in1=mu_b[:, 0:n], op=OP.subtract)
                nc.vector.tensor_mul(dst[:, c, t0:t1], dst[:, c, t0:t1],
                                     rs_b[:, 0:n])
        return rstd

    aT = apool.tile([128, 4, NTOK], BF16, tag="a", name="aT0")
    vt0 = ln_var_tile()
    mu_bs0 = [ln_stats(t0, t1, vt0, pi) for pi, (t0, t1) in enumerate(TP)]
    ln_finish(aT, mu_bs0, vt0, TP)

    for l in range(L):
        last = (l == L - 1)
        wq_t = wq_p.tile([128, 4, 3 * W], BF16, tag="wq", name=f"wq{l}")
        nc.sync.dma_start(out=wq_t,
                          in_=wqkvT_d[l].rearrange("(c p) f -> p c f", p=128))
        wo_t = wo_p.tile([128, 4, W], BF16, tag="wo", name=f"wo{l}")
        nc.sync.dma_start(out=wo_t,
                          in_=woT_d[l].rearrange("(c p) f -> p c f", p=128))
        wfc_t = wfc_p.tile([128, 4, 4 * W], BF16, tag="wfc", name=f"wfc{l}")
        nc.sync.dma_start(out=wfc_t,
                          in_=wfcT_d[l].rearrange("(c p) f -> p c f", p=128))
        wpr_t = wpr_p.tile([128, 16, W], BF16, tag="wpr", name=f"wpr{l}")
        nc.sync.dma_start(out=wpr_t,
                          in_=wprojT_d[l].rearrange("(c p) f -> p c f", p=128))

        # ---- k (all tokens) and q (frame tokens only) ----
        for t0, t1 in TP:
            n = t1 - t0
            for fc in range(4, 8):      # k chunks
                ps = psA.tile([128, 512], F32, tag="mm", name=f"k{l}_{fc}_{t0}")
                for c in range(4):
                    nc.tensor.matmul(ps[:, 0:n], wq_t[:, c, 128 * fc:128 * fc + 128],
                                     aT[:, c, t0:t1], start=(c == 0), stop=(c == 3))
                nc.vector.tensor_copy(out=qkT[:, fc, t0:t1], in_=ps[:, 0:n])
            q0 = max(t0, NNF)
            nq = t1 - q0
            for fc in range(4):         # q chunks, frame tokens only
                ps = psA.tile([128, 512], F32, tag="mm", name=f"q{l}_{fc}_{t0}")
                for c in range(4):
                    nc.tensor.matmul(ps[:, 0:nq], wq_t[:, c, 128 * fc:128 * fc + 128],
                                     aT[:, c, q0:t1], start=(c == 0), stop=(c == 3))
                nc.scalar.activation(qkT[:, fc, q0:t1], ps[:, 0:nq], AF.Copy)

        # ---- v (token-on-partition, for PV stationary) ----
        for ci, (t0, t1) in enumerate(TCH):
            rows = t1 - t0
            ps = psA.tile([128, 512], F32, tag="mm", name=f"v{l}_{ci}")
            for c in range(4):
                nc.tensor.matmul(ps[0:rows, :], aT[:, c, t0:t1],
                                 wq_t[:, c, 1024:1536], start=(c == 0), stop=(c == 3))
            nc.vector.tensor_copy(
                out=v_aug[0:rows, ci, :, 0:DH],
                in_=ps[0:rows, :].rearrange("p (hh d) -> p hh d", hh=8))

        # ---- non-frame columns of attnT = v_nf (feature-major matmul) ----
        if not last:
            for fc in range(4):
                ps = psA.tile([128, 512], F32, tag="mm", name=f"vt{l}_{fc}")
                for c in range(4):
                    nc.tensor.matmul(ps[:, 0:NNF],
                                     wq_t[:, c, 1024 + 128 * fc:1152 + 128 * fc],
                                     aT[:, c, 0:NNF], start=(c == 0), stop=(c == 3))
                nc.scalar.activation(attnT[:, fc, 0:NNF], ps[:, 0:NNF], AF.Copy)

        # ---- attention: head-pipelined (PV of head h emitted after the
        # scores of head h+1, so the mask-add/exp chain is covered by PE
        # work and the PE never waits on it) ----
        ebuf = {}

        def att_scores(h):
            r0 = 64 * (h % 2)
            qc, kc = h // 2, 4 + h // 2
            es = []
            for ci, rows in ((0, 128), (1, 52)):
                sc = psA.tile([128, 512], F32, tag="mm", name=f"snf{l}_{h}_{ci}")
                nc.tensor.matmul(sc[0:rows, :],
                                 qkT[r0:r0 + 64, kc, 128 * ci:128 * ci + rows],
                                 qkT[r0:r0 + 64, qc, NNF:NTOK],
                                 start=True, stop=True)
                nc.vector.tensor_add(out=sc[0:rows, :], in0=sc[0:rows, :],
                                     in1=maskb_t[0:rows, ci, :])
                e = ep.tile([128, 512], BF16, tag="e", name=f"e{l}_{h}_{ci}")
                nc.scalar.activation(e[0:rows, :], sc[0:rows, :], AF.Exp,
                                     scale=SCALE)
                es.append(e)
            sc = psA.tile([128, 512], F32, tag="mm", name=f"sow{l}_{h}")
            for a in range(4):
                t0 = NNF + 128 * a
                nc.tensor.matmul(sc[:, 128 * a:128 * a + 128],
                                 qkT[r0:r0 + 64, kc, t0:t0 + 128],
                                 qkT[r0:r0 + 64, qc, t0:t0 + 128],
                                 start=(a == 0), stop=(a == 3))
            e2 = eop.tile([128, 512], BF16, tag="e2", name=f"e2{l}_{h}")
            nc.scalar.activation(e2, sc, AF.Exp, scale=SCALE)
            ebuf[h] = (es, e2)

        def att_pv(h):
            r0 = 64 * (h % 2)
            es, e2 = ebuf.pop(h)
            oT = psM.tile([128, 512], F32, tag=f"g{h % 4}", name=f"oT{l}_{h}")
            oT = oT[0:65, :]
            for ci, rows in ((0, 128), (1, 52)):
                nc.tensor.matmul(oT, v_aug[0:rows, ci, h, :], es[ci][0:rows, :],
                                 start=(ci == 0), stop=False)
            for a in range(4):
                nc.tensor.matmul(oT[:, 128 * a:128 * a + 128],
                                 v_aug[:, 2 + a, h, :],
                                 e2[:, 128 * a:128 * a + 128],
                                 start=False, stop=(a == 3))
            rec = rp.tile([1, 512], F32, tag="rec", name=f"rec{l}_{h}")
            if psum_rec:
                nc.vector.reciprocal_approx_fast(out=rec, in_=oT[64:65, :])
            else:
                den = rp.tile([1, 512], F32, tag="den", name=f"den{l}_{h}")
                nc.vector.tensor_copy(out=den, in_=oT[64:65, :])
                nc.vector.reciprocal_approx_fast(out=rec, in_=den)
            dst = attnT[r0:r0 + 64, h // 2, NNF:NTOK]
            nc.scalar.activation(dst, oT[0:64, :], AF.Copy)
            if att_pb:
                rb = bc.tile([128, 512], F32, tag="rb", name=f"rb{l}_{h}")
                nc.gpsimd.partition_broadcast(rb, rec)
                nc.vector.tensor_mul(dst, dst, rb[r0:r0 + 64, :])
            else:
                rec_r = rp.tile([1, 512], F32R, tag="recr", name=f"recr{l}_{h}")
                with nc.allow_low_precision(reason="f32r broadcast operand"):
                    nc.vector.tensor_copy(out=rec_r, in_=rec)
                rb = psA.tile([128, 512], F32, tag="mm", name=f"rb{l}_{h}")
                nc.tensor.matmul(rb[0:64, :], ones_row[:, 0:64], rec_r,
                                 start=True, stop=True)
                nc.vector.tensor_mul(dst, dst, rb[0:64, :])

        if att_pipe:
            for h in range(H):
                att_scores(h)
                if h > 0:
                    att_pv(h - 1)
            att_pv(H - 1)
        else:
            for h in range(H):
                att_scores(h)
                att_pv(h)

        # ---- O-proj + residual, with LN2 stats interleaved so each pass's
        # stats/chain overlaps the next pass's matmuls ----
        passes = TPL if last else TP
        mT = apool.tile([128, 4, NTOK], BF16, tag="a", name=f"mT{l}")
        vt2 = ln_var_tile()
        mu_bs2 = []
        for pi, (t0, t1) in enumerate(passes):
            n = t1 - t0
            for fc in range(4):
                ps = psA.tile([128, 512], F32, tag="mm", name=f"op{l}_{fc}_{t0}")
                for c in range(4):
                    nc.tensor.matmul(ps[:, 0:n], wo_t[:, c, 128 * fc:128 * fc + 128],
                                     attnT[:, c, t0:t1], start=(c == 0), stop=(c == 3))
                nc.vector.tensor_add(out=hT[:, fc, t0:t1], in0=hT[:, fc, t0:t1],
                                     in1=ps[:, 0:n])
            mu_bs2.append(ln_stats(t0, t1, vt2, pi))
        rstd2 = ln_finish(mT, mu_bs2, vt2, passes)
        # preload the gelu table; depends on the batched Exp so the load slots
        # in right after the LN2 scalar work (before the first real gelu)
        dmy = sp.tile([1, 346], F32, tag="dmy", name=f"dmy{l}")
        nc.scalar.activation(dmy[0:1, 0:1], rstd2[0:1, 0, 0:1], AF.Gelu)
        st1 = {}
        vt1 = ln_var_tile() if not last else None
        for pi, (t0, t1) in enumerate(passes):
            n = t1 - t0
            acc = [psM.tile([128, 512], F32, tag=f"g{i}", name=f"acc{l}_{t0}_{i}")
                   for i in range(4)]
            for uc in range(16):
                ups = psA.tile([128, 512], F32, tag="mm", name=f"u{l}_{t0}_{uc}")
                for c in range(4):
                    nc.tensor.matmul(ups[:, 0:n], wfc_t[:, c, 128 * uc:128 * uc + 128],
                                     mT[:, c, t0:t1], start=(c == 0), stop=(c == 3))
                ug = up.tile([128, 346], BF16, tag="ug", name=f"ug{l}_{t0}_{uc}")
                nc.scalar.activation(ug[:, 0:n], ups[:, 0:n], AF.Gelu)
                for fc in range(4):
                    nc.tensor.matmul(acc[fc][:, 0:n],
                                     wpr_t[:, uc, 128 * fc:128 * fc + 128],
                                     ug[:, 0:n], start=(uc == 0), stop=(uc == 15))
            for fc in range(4):
                nc.vector.tensor_add(out=hT[:, fc, t0:t1], in0=hT[:, fc, t0:t1],
                                     in1=acc[fc][:, 0:n])
            # LN1 stats of the next layer for this token pass: the PE matmuls
            # and vector chain run while the other pass's MLP streams.
            if not last:
                st1[pi] = ln_stats(TP[pi][0], TP[pi][1], vt1, pi)
        # switch the table back via Ln (Ln lives only in natural_log_exp, so
        # walrus loads the set that also covers the attention/LN exps);
        # reading the last ug makes the load wait for the final gelu
        dmy2 = sp.tile([1, 346], F32, tag="dmy2", name=f"dmy2{l}")
        nc.scalar.activation(dmy2[0:1, 0:1], ug[0:1, 0:1], AF.Ln, bias=cvec_t[0:1, 1:2])
        if not last:
            aT = apool.tile([128, 4, NTOK], BF16, tag="a", name=f"aT{l + 1}")
            ln_finish(aT, [st1.pop(0), st1.pop(1)], vt1, TP)

    # ---- logits for frame tokens ----
    pred_t = wfc_p.tile([128, 4, 4 * W], F32R, tag="wfc", name="pred_t")
    nc.sync.dma_start(out=pred_t[:, :, 0:1024],
                      in_=predT_d.rearrange("(c p) f -> p c f", p=128).bitcast(F32R))
    for k in range(4):
        for nb in range(2):
            ps = psA.tile([128, 512], F32, tag="mm", name=f"lg{k}_{nb}")
            for c in range(4):
                nc.tensor.matmul(ps, hT[:, c, NNF + 128 * k:NNF + 128 * k + 128],
                                 pred_t[:, c, 512 * nb:512 * nb + 512],
                                 start=(c == 0), stop=(c == 3))
            lo = lout.tile([128, 512], F32, tag="lo", name=f"lo{k}_{nb}")
            nc.vector.tensor_copy(out=lo, in_=ps)
            nc.sync.dma_start(out=out_d[128 * k:128 * k + 128, 512 * nb:512 * nb + 512],
                              in_=lo)

    ctx.close()


# (waitsplit embedded so kernel.py is self-contained)
import types as _types
waitsplit_embedded = _types.ModuleType("waitsplit_embedded")


def _split_excess_waits(nc):
    n_split = 0
    for fn in nc.m.functions:
        for bb in fn.blocks:
            insts = list(bb.instructions)
            new_list = []
            changed = False
            for inst in insts:
                si = getattr(inst, "sync_info", None)
                waits = list(si.on_wait) if si is not None and si.on_wait else []
                cap = 2 if isinstance(inst, mybir.InstEventSemaphore) else 1
                if len(waits) > cap:
                    changed = True
                    keep = waits[-cap:]
                    for w in waits[:-cap]:
                        n_split += 1
                        nop = mybir.InstNoOp(
                            name=f"WSPLIT-{n_split}-{inst.name}",
                            engine=inst.engine,
                            ins=[], outs=[],
                            sync_info=mybir.SyncInfo(on_wait=[w], on_update=[]),
                        )
                        try:
                            nop.bass_nofuse = True
                        except Exception:
                            pass
                        new_list.append(nop)
                    inst.sync_info = mybir.SyncInfo(on_wait=keep,
                                                    on_update=list(si.on_update))
                new_list.append(inst)
            if changed:
                try:
                    bb.instructions = new_list
                except Exception:
                    bb.instructions.clear()
                    bb.instructions.extend(new_list)
    return n_split


waitsplit_embedded.split_excess_waits = _split_excess_waits
sys.modules["waitsplit_embedded"] = waitsplit_embedded


# ---------------- host side ----------------

def _sinusoidal_pos_emb(n_pos, d, n=10000.0):
    pos = np.arange(n_pos, dtype=np.float32)[:, None]
    den = np.power(n, 2.0 * np.arange(d // 2, dtype=np.float32) / d).astype(np.float32)
    emb = np.zeros((n_pos, d), dtype=np.float32)
    emb[:, 0::2] = np.sin(pos / den)
    emb[:, 1::2] = np.cos(pos / den)
    return emb


_PROG = None


def kernel(**inputs):
    global _PROG
    x = np.ascontiguousarray(np.asarray(inputs["x"], dtype=np.float32))
    f = np.ascontiguousarray(np.asarray(inputs["f"], dtype=np.float32))
    delim = np.asarray(inputs["frame_delim"], dtype=np.float32)
    wqkv = np.asarray(inputs["wqkv"], dtype=np.float32)
    wo = np.asarray(inputs["wo"], dtype=np.float32)
    wfc = np.asarray(inputs["wfc"], dtype=np.float32)
    wproj = np.asarray(inputs["wproj"], dtype=np.float32)
    pred_w = np.asarray(inputs["pred_w"], dtype=np.float32)

    # this kernel folds away the (identity) LN affine and (zero) biases;
    # verify that assumption against the actual inputs
    assert np.all(np.asarray(inputs["ln1_g"]) == 1), "nonconst ln1_g"
    assert np.all(np.asarray(inputs["ln2_g"]) == 1), "nonconst ln2_g"
    assert np.all(np.asarray(inputs["ln1_b"]) == 0), "nonzero ln1_b"
    assert np.all(np.asarray(inputs["ln2_b"]) == 0), "nonzero ln2_b"
    for bname in ("bqkv", "bo", "bfc", "bproj"):
        assert np.all(np.asarray(inputs[bname]) == 0), f"nonzero {bname}"

    d2 = np.broadcast_to(delim, (B, N, 1, W))
    fx = np.concatenate([x, d2, f, d2], axis=-2).reshape(B, S, W)
    fx = fx + _sinusoidal_pos_emb(S, W)[None]

    nf_idx = (np.arange(N)[:, None] * BLK + (F + np.arange(T + 2))[None, :]).reshape(-1)
    jj = np.arange(NNF) // (T + 2)
    rr = np.arange(NNF) % (T + 2)
    mask = np.full((NNF, N), NEG, np.float32)
    for i in range(N):
        allowed = ((rr <= T) & (jj <= i)) | ((rr == T + 1) & (jj == i - 1))
        mask[allowed, i] = 0.0

    bf = ml_dtypes.bfloat16
    wqkvT = np.ascontiguousarray(wqkv.transpose(0, 2, 1)).astype(bf)
    woT = np.ascontiguousarray(wo.transpose(0, 2, 1)).astype(bf)
    wfcT = np.ascontiguousarray(wfc.transpose(0, 2, 1)).astype(bf)
    wprojT = np.ascontiguousarray(wproj.transpose(0, 2, 1)).astype(bf)
    predT = np.ascontiguousarray(pred_w.T)

    cvec = np.zeros((128, 4), np.float32)
    cvec[:, 0] = 1.0 / W
    cvec[:, 1] = 1.0
    cvec[:, 2] = EPS

    if _PROG is None:
        import os
        _PROG = build_program(ln_pb=os.environ.get("LN_PB", "1") == "1",
                              att_pb=os.environ.get("ATT_PB", "1") == "1",
                              att_pipe=os.environ.get("ATT_PIPE", "1") == "1",
                              psum_rec=os.environ.get("PSUM_REC", "0") == "1")
    nc = _PROG

    in_maps = []
    for c in range(8):
        b, slot = c // 4, c % 4
        cf = CORE_FRAMES[slot]
        fr_idx = np.concatenate([np.arange(i * BLK, i * BLK + F) for i in cf])
        tok = np.concatenate([nf_idx, fr_idx])
        h0T = np.ascontiguousarray(fx[b, tok, :].T)
        in_maps.append({
            "h0": h0T,
            "wqkvT": wqkvT, "woT": woT, "wfcT": wfcT, "wprojT": wprojT,
            "predT": predT,
            "nfmask": np.ascontiguousarray(np.repeat(mask[:, cf], F, axis=1)),
            "cvec": cvec,
        })

    res = run_bass_kernel_spmd(nc, in_maps, list(range(8)))

    out = np.zeros((B, N, F, 1024), np.float32)
    for c in range(8):
        b, slot = c // 4, c % 4
        lo = res.results[c]["logits"].reshape(4, F, 1024)
        for si, i in enumerate(CORE_FRAMES[slot]):
            if slot == 3 and si == 0:
                continue
            out[b, i] = lo[si]
    return out.reshape(B, N * F, 1024)


# revision 24
# speedup vs baseline: 1.0270x; 1.0270x over previous
"""Trainium2 Bass kernel for nn_Decoder_75892072120909 (sparse-attention decoder).

Self-contained: takes FULL inputs (as produced by the problem's setup_inputs),
runs an 8-core SPMD Bass kernel, returns the FULL output [2, 1920, 1024].

Sharding: 2 batches x 4 cores; each core owns 4 frame blocks (the last core of
a batch owns frames [11,12,13,14]; frame 11 is taken from the previous core so
every core runs the identical SPMD program). Each core also replicates the
tiny "non-frame" token trajectory (delim + dynamics tokens, 12 per block = 180
per batch) whose attention is the identity (those tokens attend only to
themselves), so no cross-core communication is needed.

On-core layout: activations are feature-on-partition ("transposed world",
hT [512, Ntok]) so every matmul consumes operands natively. The residual
stream hT stays fp32(r); everything downstream of the LayerNorms (LN outputs,
q/k/v, exp'd scores, gelu outputs and all weights) is bf16, which keeps every
matmul at 1 cycle/row including the N=128 frame-attention blocks (f32r drops
to 4 cycles/row below N=256). Softmax denominators use the single-instruction
DVE reciprocal_approx_fast (~18 bits) instead of the 8-cycle iterative
reciprocal. Non-frame tokens get no q (they are never attention queries) and
in the last layer skip o-proj/LN2/MLP (their logits are discarded).
LayerNorm statistics use ones-vector matmuls on the tensor engine;
rstd = exp(-0.5*ln(var+eps)) keeps the scalar engine on the natural_log_exp
table set shared with the attention exponentials. LN1 of layer l+1 is emitted
inside layer l's MLP tail so its stats/broadcast overlap the down-proj stream.
"""

import sys
import numpy as np

for _p in ("/opt/trn_rl_repo", "/root/.axon_site/_ro/trn_rl_repo"):
    if _p not in sys.path:
        sys.path.append(_p)

import ml_dtypes
import concourse.bass as bass
import concourse.tile as tile
from concourse import mybir
from concourse.bass_utils import run_bass_kernel_spmd

# ---------------- problem constants (hardcoded) ----------------
F = 128           # frame tokens per block
T = 10            # dynamics tokens per block
BLK = F + T + 2   # 140
N = 15            # frame blocks
B = 2
W = 512
L = 8
H = 8
DH = 64
S = N * BLK       # 2100
NNF = N * (T + 2)  # 180 non-frame tokens per batch
NQ = 4 * F        # 512 frame-token queries per core
NTOK = NNF + NQ   # 692 tokens per core
EPS = 1e-5
NEG = -1e30
SCALE = 1.0 / np.sqrt(DH)
CORE_FRAMES = [[0, 1, 2, 3], [4, 5, 6, 7], [8, 9, 10, 11], [11, 12, 13, 14]]
F32 = mybir.dt.float32
F32R = mybir.dt.float32r
BF16 = mybir.dt.bfloat16
AF = mybir.ActivationFunctionType
OP = mybir.AluOpType
TP = ((0, 346), (346, NTOK))        # full token passes
TPL = ((NNF, 436), (436, NTOK))     # last-layer passes (frame tokens only)
# attention-aligned token chunks: 0-1 = the 180 nf tokens, 2-5 = frame tokens
TCH = ((0, 128), (128, 180), (180, 308), (308, 436), (436, 564), (564, 692))


def build_program(waitsplit=True, ln_pb=True, att_pb=True, att_pipe=True, psum_rec=False):
    nc = bass.Bass("TRN2", target_bir_lowering=False, debug=False, num_devices=8)

    h0 = nc.dram_tensor("h0", [W, NTOK], F32, kind="ExternalInput").ap()
    wqkvT_d = nc.dram_tensor("wqkvT", [L, W, 3 * W], BF16, kind="ExternalInput").ap()
    woT_d = nc.dram_tensor("woT", [L, W, W], BF16, kind="ExternalInput").ap()
    wfcT_d = nc.dram_tensor("wfcT", [L, W, 4 * W], BF16, kind="ExternalInput").ap()
    wprojT_d = nc.dram_tensor("wprojT", [L, 4 * W, W], BF16, kind="ExternalInput").ap()
    predT_d = nc.dram_tensor("predT", [W, 1024], F32, kind="ExternalInput").ap()
    mask_d = nc.dram_tensor("nfmask", [NNF, 512], F32, kind="ExternalInput").ap()
    cvec_d = nc.dram_tensor("cvec", [128, 4], F32, kind="ExternalInput").ap()
    out_d = nc.dram_tensor("logits", [NQ, 1024], F32, kind="ExternalOutput").ap()

    with tile.TileContext(nc) as tc:
        _build(tc, h0, wqkvT_d, woT_d, wfcT_d, wprojT_d, predT_d, mask_d,
               cvec_d, out_d, ln_pb, att_pb, att_pipe, psum_rec)

    from concourse.library_overlay import lower_extended_insts
    lower_extended_insts(nc)
    if waitsplit:
        from waitsplit_embedded import split_excess_waits
        split_excess_waits(nc)
    return nc


def _build(tc, h0, wqkvT_d, woT_d, wfcT_d, wprojT_d, predT_d, mask_d,
           cvec_d, out_d, ln_pb=True, att_pb=True, att_pipe=True, psum_rec=True):
    nc = tc.nc
    from contextlib import ExitStack
    ctx = ExitStack()

    def pool(name, bufs, **kw):
        return ctx.enter_context(tc.tile_pool(name=name, bufs=bufs, **kw))

    state = pool("state", 1)
    apool = pool("apool", 3)
    qkp = pool("qkp", 1)
    vp = pool("vp", 1)
    attp = pool("attp", 1)
    ep = pool("ep", 5)
    eop = pool("eop", 3)
    up = pool("up", 3)
    sq = pool("sq", 4)
    sp = pool("sp", 2)
    rp = pool("rp", 2)
    bc = pool("bc", 2)
    wq_p = pool("wq", 1)
    wo_p = pool("wo", 1)
    wfc_p = pool("wfc", 1)
    wpr_p = pool("wpr", 1)
    cst = pool("cst", 1)
    lout = pool("lout", 2)

    # PSUM: 8 banks. psA(mm)x4 rotate through everything transient (LN stats,
    # qkv/v/score/o-proj/up psums); psM g0..g3 are the long-lived accumulators
    # (per-head attention oT during attention, MLP down-proj acc during the
    # MLP). All broadcasts go through gpsimd partition_broadcast into SBUF so
    # the tensor engine never sits behind a scalar/vector chain.
    psA = pool("psA", 4, space="PSUM")
    psM = pool("psM", 1, space="PSUM")

    # ---- constants ----
    ones_inv = cst.tile([128, 1], F32R, name="ones_inv")   # value 1/512
    nc.sync.dma_start(out=ones_inv, in_=cvec_d[:, 0:1].bitcast(F32R))
    cvec_t = cst.tile([128, 4], F32, name="cvec_t")
    nc.sync.dma_start(out=cvec_t, in_=cvec_d)
    ones_row = cst.tile([1, 128], F32R, name="ones_row")
    nc.sync.dma_start(out=ones_row, in_=cvec_d[:, 1:2].rearrange("p o -> o p").bitcast(F32R))
    maskb_t = cst.tile([128, 2, 512], F32, name="maskb_t")
    nc.sync.dma_start(out=maskb_t[0:128, 0, :], in_=mask_d[0:128, :])
    nc.sync.dma_start(out=maskb_t[0:52, 1, :], in_=mask_d[128:180, :])

    # ---- persistent activations ----
    hT = state.tile([128, 4, NTOK], F32R, name="hT")
    nc.sync.dma_start(out=hT, in_=h0.rearrange("(c p) t -> p c t", p=128).bitcast(F32R))
    qkT = qkp.tile([128, 8, NTOK], BF16, name="qkT")
    v_aug = vp.tile([128, 6, H, DH + 1], BF16, name="v_aug")
    for ci in range(6):
        nc.gpsimd.memset(v_aug[:, ci, :, DH:DH + 1], 1.0)
    attnT = attp.tile([128, 4, NTOK], BF16, name="attnT")

    # partition_broadcast lives in the Q7 "proxy" ucode library
    from concourse import library_config
    nc.gpsimd.load_library(library_config.proxy)

    def ln_stats(t0, t1, vt, row):
        """PE stats matmuls + vector mu/var chain for tokens [t0,t1).
        Writes the variance into row `row` of the shared tile `vt` so the
        scalar Ln/Exp of both passes run as ONE pair of instructions (keeps
        the act-table flips at exactly 2 per layer)."""
        n = t1 - t0
        mu_ps = psA.tile([1, 512], F32, tag="mm", name=f"mu{nc.next_id()}")
        ms_ps = psA.tile([1, 512], F32, tag="mm", name=f"ms{nc.next_id()}")
        for c in range(4):
            nc.tensor.matmul(mu_ps[:, 0:n], ones_inv, hT[:, c, t0:t1],
                             start=(c == 0), stop=(c == 3))
        for c in range(4):
            hsq = sq.tile([128, 346], F32R, tag="hsq", name=f"hsq{nc.next_id()}")
            nc.gpsimd.tensor_mul(hsq[:, 0:n], hT[:, c, t0:t1], hT[:, c, t0:t1])
            nc.tensor.matmul(ms_ps[:, 0:n], ones_inv, hsq[:, 0:n],
                             start=(c == 0), stop=(c == 3))
        mu = sp.tile([1, 346], F32R, tag="mu", name=f"muv{nc.next_id()}")
        nc.vector.tensor_copy(out=mu[:, 0:n], in_=mu_ps[:, 0:n])
        # broadcast of mu can start as soon as the copy lands
        mu_b = bc.tile([128, 346], F32R, tag="mu_b", name=f"mub{nc.next_id()}")
        nc.gpsimd.partition_broadcast(mu_b[:, 0:n], mu[:, 0:n])
        musq = sp.tile([1, 346], F32, tag="musq", name=f"mq{nc.next_id()}")
        nc.vector.tensor_mul(musq[:, 0:n], mu[:, 0:n], mu[:, 0:n])
        nc.vector.tensor_tensor(out=vt[0:1, row, 0:n], in0=ms_ps[:, 0:n],
                                in1=musq[:, 0:n], op=OP.subtract)
        return mu_b

    def ln_var_tile():
        # both passes' variances batched along the free dim of one partition
        return sp.tile([1, 2, 346], F32, tag="var", name=f"var{nc.next_id()}")

    def ln_finish1(dst, mu_b, vt, row, t0, t1):
        """per-pass Ln+Exp chain (pipelines against the other pass's matmuls)."""
        n = t1 - t0
        lnv = sp.tile([1, 2, 346], F32, tag="lnv", name=f"lnv{nc.next_id()}")
        nc.scalar.activation(lnv[0:1, row, 0:n], vt[0:1, row, 0:n], AF.Ln,
                             bias=cvec_t[0:1, 2:3])
        rstd = sp.tile([1, 2, 346], F32R, tag="rstd", name=f"rsd{nc.next_id()}")
        nc.scalar.activation(rstd[0:1, row, 0:n], lnv[0:1, row, 0:n],
                             AF.Exp, scale=-0.5)
        rs_b = bc.tile([128, 346], F32R, tag="rs_b", name=f"rsb{nc.next_id()}")
        nc.gpsimd.partition_broadcast(rs_b[:, 0:n], rstd[0:1, row, 0:n])
        for c in range(4):
            nc.vector.tensor_tensor(out=dst[:, c, t0:t1], in0=hT[:, c, t0:t1],
                                    in1=mu_b[:, 0:n], op=OP.subtract)
            nc.vector.tensor_mul(dst[:, c, t0:t1], dst[:, c, t0:t1],
                                 rs_b[:, 0:n])
        return rstd

    def ln_finish(dst, mu_bs, vt, passes):
        """one batched Ln+Exp for both passes, then per-pass bcast + apply."""
        nn = passes[0][1] - passes[0][0]
        assert all(t1 - t0 == nn for t0, t1 in passes)
        lnv = sp.tile([1, 2, 346], F32, tag="lnv", name=f"lnv{nc.next_id()}")
        nc.scalar.activation(lnv[:, :, 0:nn], vt[:, :, 0:nn], AF.Ln,
                             bias=cvec_t[0:1, 2:3])
        rstd = sp.tile([1, 2, 346], F32R, tag="rstd", name=f"rsd{nc.next_id()}")
        nc.scalar.activation(rstd[:, :, 0:nn], lnv[:, :, 0:nn],
                             AF.Exp, scale=-0.5)
        for pi, (t0, t1) in enumerate(passes):
            n = t1 - t0
            rs_b = bc.tile([128, 346], F32R, tag="rs_b", name=f"rsb{nc.next_id()}")
            nc.gpsimd.partition_broadcast(rs_b[:, 0:n], rstd[0:1, pi, 0:n])
            mu_b = mu_bs[pi]
            for c in range(4):
                nc.vector.tensor_tensor(out=dst[:, c, t0:t1], in0=hT[:, c, t0:t1],
                                        in1=mu_b[:, 0:n], op=OP.subtract)
                nc.vector.tensor_mul(dst[:, c, t0:t1], dst[:, c, t0:t1],
                                     rs_b[:, 0:n])
        return rstd

    aT = apool.tile([128, 4, NTOK], BF16, tag="a", name="aT0")
    vt0 = ln_var_tile()
    mu_bs0 = [ln_stats(t0, t1, vt0, pi) for pi, (t0, t1) in enumerate(TP)]
    ln_finish(aT, mu_bs0, vt0, TP)

    for l in range(L):
        last = (l == L - 1)
        wq_t = wq_p.tile([128, 4, 3 * W], BF16, tag="wq", name=f"wq{l}")
        nc.sync.dma_start(out=wq_t,
                          in_=wqkvT_d[l].rearrange("(c p) f -> p c f", p=128))
        wo_t = wo_p.tile([128, 4, W], BF16, tag="wo", name=f"wo{l}")
        nc.sync.dma_start(out=wo_t,
                          in_=woT_d[l].rearrange("(c p) f -> p c f", p=128))
        wfc_t = wfc_p.tile([128, 4, 4 * W], BF16, tag="wfc", name=f"wfc{l}")
        nc.sync.dma_start(out=wfc_t,
                          in_=wfcT_d[l].rearrange("(c p) f -> p c f", p=128))
        wpr_t = wpr_p.tile([128, 16, W], BF16, tag="wpr", name=f"wpr{l}")
        nc.sync.dma_start(out=wpr_t,
                          in_=wprojT_d[l].rearrange("(c p) f -> p c f", p=128))

        # ---- k (all tokens) and q (frame tokens only) ----
        for t0, t1 in TP:
            n = t1 - t0
            for fc in range(4, 8):      # k chunks
                ps = psA.tile([128, 512], F32, tag="mm", name=f"k{l}_{fc}_{t0}")
                for c in range(4):
                    nc.tensor.matmul(ps[:, 0:n], wq_t[:, c, 128 * fc:128 * fc + 128],
                                     aT[:, c, t0:t1], start=(c == 0), stop=(c == 3))
                nc.vector.tensor_copy(out=qkT[:, fc, t0:t1], in_=ps[:, 0:n])
            q0 = max(t0, NNF)
            nq = t1 - q0
            for fc in range(4):         # q chunks, frame tokens only
                ps = psA.tile([128, 512], F32, tag="mm", name=f"q{l}_{fc}_{t0}")
                for c in range(4):
                    nc.tensor.matmul(ps[:, 0:nq], wq_t[:, c, 128 * fc:128 * fc + 128],
                                     aT[:, c, q0:t1], start=(c == 0), stop=(c == 3))
                nc.scalar.activation(qkT[:, fc, q0:t1], ps[:, 0:nq], AF.Copy)

        # ---- v (token-on-partition, for PV stationary) ----
        for ci, (t0, t1) in enumerate(TCH):
            rows = t1 - t0
            ps = psA.tile([128, 512], F32, tag="mm", name=f"v{l}_{ci}")
            for c in range(4):
                nc.tensor.matmul(ps[0:rows, :], aT[:, c, t0:t1],
                                 wq_t[:, c, 1024:1536], start=(c == 0), stop=(c == 3))
            nc.vector.tensor_copy(
                out=v_aug[0:rows, ci, :, 0:DH],
                in_=ps[0:rows, :].rearrange("p (hh d) -> p hh d", hh=8))

        # ---- non-frame columns of attnT = v_nf (feature-major matmul) ----
        if not last:
            for fc in range(4):
                ps = psA.tile([128, 512], F32, tag="mm", name=f"vt{l}_{fc}")
                for c in range(4):
                    nc.tensor.matmul(ps[:, 0:NNF],
                                     wq_t[:, c, 1024 + 128 * fc:1152 + 128 * fc],
                                     aT[:, c, 0:NNF], start=(c == 0), stop=(c == 3))
                nc.scalar.activation(attnT[:, fc, 0:NNF], ps[:, 0:NNF], AF.Copy)

        # ---- attention: head-pipelined (PV of head h emitted after the
        # scores of head h+1, so the mask-add/exp chain is covered by PE
        # work and the PE never waits on it) ----
        ebuf = {}

        def att_scores(h):
            r0 = 64 * (h % 2)
            qc, kc = h // 2, 4 + h // 2
            es = []
            for ci, rows in ((0, 128), (1, 52)):
                sc = psA.tile([128, 512], F32, tag="mm", name=f"snf{l}_{h}_{ci}")
                nc.tensor.matmul(sc[0:rows, :],
                                 qkT[r0:r0 + 64, kc, 128 * ci:128 * ci + rows],
                                 qkT[r0:r0 + 64, qc, NNF:NTOK],
                                 start=True, stop=True)
                nc.vector.tensor_add(out=sc[0:rows, :], in0=sc[0:rows, :],
                                     in1=maskb_t[0:rows, ci, :])
                e = ep.tile([128, 512], BF16, tag="e", name=f"e{l}_{h}_{ci}")
                nc.scalar.activation(e[0:rows, :], sc[0:rows, :], AF.Exp,
                                     scale=SCALE)
                es.append(e)
            sc = psA.tile([128, 512], F32, tag="mm", name=f"sow{l}_{h}")
            for a in range(4):
                t0 = NNF + 128 * a
                nc.tensor.matmul(sc[:, 128 * a:128 * a + 128],
                                 qkT[r0:r0 + 64, kc, t0:t0 + 128],
                                 qkT[r0:r0 + 64, qc, t0:t0 + 128],
                                 start=(a == 0), stop=(a == 3))
            e2 = eop.tile([128, 512], BF16, tag="e2", name=f"e2{l}_{h}")
            nc.scalar.activation(e2, sc, AF.Exp, scale=SCALE)
            ebuf[h] = (es, e2)

        def att_pv(h):
            r0 = 64 * (h % 2)
            es, e2 = ebuf.pop(h)
            oT = psM.tile([128, 512], F32, tag=f"g{h % 4}", name=f"oT{l}_{h}")
            oT = oT[0:65, :]
            for ci, rows in ((0, 128), (1, 52)):
                nc.tensor.matmul(oT, v_aug[0:rows, ci, h, :], es[ci][0:rows, :],
                                 start=(ci == 0), stop=False)
            for a in range(4):
                nc.tensor.matmul(oT[:, 128 * a:128 * a + 128],
                                 v_aug[:, 2 + a, h, :],
                                 e2[:, 128 * a:128 * a + 128],
                                 start=False, stop=(a == 3))
            rec = rp.tile([1, 512], F32, tag="rec", name=f"rec{l}_{h}")
            if psum_rec:
                nc.vector.reciprocal_approx_fast(out=rec, in_=oT[64:65, :])
            else:
                den = rp.tile([1, 512], F32, tag="den", name=f"den{l}_{h}")
                nc.vector.tensor_copy(out=den, in_=oT[64:65, :])
                nc.vector.reciprocal_approx_fast(out=rec, in_=den)
            dst = attnT[r0:r0 + 64, h // 2, NNF:NTOK]
            nc.scalar.activation(dst, oT[0:64, :], AF.Copy)
            if att_pb:
                rb = bc.tile([128, 512], F32, tag="rb", name=f"rb{l}_{h}")
                nc.gpsimd.partition_broadcast(rb, rec)
                nc.vector.tensor_mul(dst, dst, rb[r0:r0 + 64, :])
            else:
                rec_r = rp.tile([1, 512], F32R, tag="recr", name=f"recr{l}_{h}")
                with nc.allow_low_precision(reason="f32r broadcast operand"):
                    nc.vector.tensor_copy(out=rec_r, in_=rec)
                rb = psA.tile([128, 512], F32, tag="mm", name=f"rb{l}_{h}")
                nc.tensor.matmul(rb[0:64, :], ones_row[:, 0:64], rec_r,
                                 start=True, stop=True)
                nc.vector.tensor_mul(dst, dst, rb[0:64, :])

        if att_pipe:
            for h in range(H):
                att_scores(h)
                if h > 0:
                    att_pv(h - 1)
            att_pv(H - 1)
        else:
            for h in range(H):
                att_scores(h)
                att_pv(h)

        # ---- O-proj + residual, with LN2 stats interleaved so each pass's
        # stats/chain overlaps the next pass's matmuls ----
        passes = TPL if last else TP
        mT = apool.tile([128, 4, NTOK], BF16, tag="a", name=f"mT{l}")
        vt2 = ln_var_tile()
        mu_bs2 = []
        for pi, (t0, t1) in enumerate(passes):
            n = t1 - t0
            for fc in range(4):
                ps = psA.tile([128, 512], F32, tag="mm", name=f"op{l}_{fc}_{t0}")
                for c in range(4):
                    nc.tensor.matmul(ps[:, 0:n], wo_t[:, c, 128 * fc:128 * fc + 128],
                                     attnT[:, c, t0:t1], start=(c == 0), stop=(c == 3))
                nc.vector.tensor_add(out=hT[:, fc, t0:t1], in0=hT[:, fc, t0:t1],
                                     in1=ps[:, 0:n])
            mu_bs2.append(ln_stats(t0, t1, vt2, pi))
            if pi > 0:
                ln_finish1(mT, mu_bs2[pi - 1], vt2, pi - 1, *passes[pi - 1])
        rstd2 = ln_finish1(mT, mu_bs2[-1], vt2, len(passes) - 1, *passes[-1])
        # preload the gelu table; depends on the last LN2 Exp so the load
        # slots in right after the LN2 scalar work
        dmy = sp.tile([1, 346], F32, tag="dmy", name=f"dmy{l}")
        nc.scalar.activation(dmy[0:1, 0:1], rstd2[0:1, 1, 0:1], AF.Gelu)
        st1 = {}
        vt1 = ln_var_tile() if not last else None
        for pi, (t0, t1) in enumerate(passes):
            n = t1 - t0
            acc = [psM.tile([128, 512], F32, tag=f"g{i}", name=f"acc{l}_{t0}_{i}")
                   for i in range(4)]
            for uc in range(16):
                ups = psA.tile([128, 512], F32, tag="mm", name=f"u{l}_{t0}_{uc}")
                for c in range(4):
                    nc.tensor.matmul(ups[:, 0:n], wfc_t[:, c, 128 * uc:128 * uc + 128],
                                     mT[:, c, t0:t1], start=(c == 0), stop=(c == 3))
                ug = up.tile([128, 346], BF16, tag="ug", name=f"ug{l}_{t0}_{uc}")
                nc.scalar.activation(ug[:, 0:n], ups[:, 0:n], AF.Gelu)
                for fc in range(4):
                    nc.tensor.matmul(acc[fc][:, 0:n],
                                     wpr_t[:, uc, 128 * fc:128 * fc + 128],
                                     ug[:, 0:n], start=(uc == 0), stop=(uc == 15))
            for fc in range(4):
                nc.vector.tensor_add(out=hT[:, fc, t0:t1], in0=hT[:, fc, t0:t1],
                                     in1=acc[fc][:, 0:n])
            # LN1 stats of the next layer for this token pass: the PE matmuls
            # and vector chain run while the other pass's MLP streams.
            if not last:
                st1[pi] = ln_stats(TP[pi][0], TP[pi][1], vt1, pi)
        # switch the table back via Ln (Ln lives only in natural_log_exp, so
        # walrus loads the set that also covers the attention/LN exps);
        # reading the last ug makes the load wait for the final gelu
        dmy2 = sp.tile([1, 346], F32, tag="dmy2", name=f"dmy2{l}")
        nc.scalar.activation(dmy2[0:1, 0:1], ug[0:1, 0:1], AF.Ln, bias=cvec_t[0:1, 1:2])
        if not last:
            aT = apool.tile([128, 4, NTOK], BF16, tag="a", name=f"aT{l + 1}")
            ln_finish(aT, [st1.pop(0), st1.pop(1)], vt1, TP)

    # ---- logits for frame tokens ----
    pred_t = wfc_p.tile([128, 4, 4 * W], F32R, tag="wfc", name="pred_t")
    nc.sync.dma_start(out=pred_t[:, :, 0:1024],
                      in_=predT_d.rearrange("(c p) f -> p c f", p=128).bitcast(F32R))
    for k in range(4):
        for nb in range(2):
            ps = psA.tile([128, 512], F32, tag="mm", name=f"lg{k}_{nb}")
            for c in range(4):
                nc.tensor.matmul(ps, hT[:, c, NNF + 128 * k:NNF + 128 * k + 128],
                                 pred_t[:, c, 512 * nb:512 * nb + 512],
                                 start=(c == 0), stop=(c == 3))
            lo = lout.tile([128, 512], F32, tag="lo", name=f"lo{k}_{nb}")
            nc.vector.tensor_copy(out=lo, in_=ps)
            nc.sync.dma_start(out=out_d[128 * k:128 * k + 128, 512 * nb:512 * nb + 512],
                              in_=lo)

    ctx.close()


# (waitsplit embedded so kernel.py is self-contained)
import types as _types
waitsplit_embedded = _types.ModuleType("waitsplit_embedded")


def _split_excess_waits(nc):
    n_split = 0
    for fn in nc.m.functions:
        for bb in fn.blocks:
            insts = list(bb.instructions)
            new_list = []
            changed = False
            for inst in insts:
                si = getattr(inst, "sync_info", None)
                waits = list(si.on_wait) if si is not None and si.on_wait else []
                cap = 2 if isinstance(inst, mybir.InstEventSemaphore) else 1
                if len(waits) > cap:
                    changed = True
                    keep = waits[-cap:]
                    for w in waits[:-cap]:
                        n_split += 1
                        nop = mybir.InstNoOp(
                            name=f"WSPLIT-{n_split}-{inst.name}",
                            engine=inst.engine,
                            ins=[], outs=[],
                            sync_info=mybir.SyncInfo(on_wait=[w], on_update=[]),
                        )
                        try:
                            nop.bass_nofuse = True
                        except Exception:
                            pass
                        new_list.append(nop)
                    inst.sync_info = mybir.SyncInfo(on_wait=keep,
                                                    on_update=list(si.on_update))
                new_list.append(inst)
            if changed:
                try:
                    bb.instructions = new_list
                except Exception:
                    bb.instructions.clear()
                    bb.instructions.extend(new_list)
    return n_split


waitsplit_embedded.split_excess_waits = _split_excess_waits
sys.modules["waitsplit_embedded"] = waitsplit_embedded


# ---------------- host side ----------------

def _sinusoidal_pos_emb(n_pos, d, n=10000.0):
    pos = np.arange(n_pos, dtype=np.float32)[:, None]
    den = np.power(n, 2.0 * np.arange(d // 2, dtype=np.float32) / d).astype(np.float32)
    emb = np.zeros((n_pos, d), dtype=np.float32)
    emb[:, 0::2] = np.sin(pos / den)
    emb[:, 1::2] = np.cos(pos / den)
    return emb


_PROG = None


def kernel(**inputs):
    global _PROG
    x = np.ascontiguousarray(np.asarray(inputs["x"], dtype=np.float32))
    f = np.ascontiguousarray(np.asarray(inputs["f"], dtype=np.float32))
    delim = np.asarray(inputs["frame_delim"], dtype=np.float32)
    wqkv = np.asarray(inputs["wqkv"], dtype=np.float32)
    wo = np.asarray(inputs["wo"], dtype=np.float32)
    wfc = np.asarray(inputs["wfc"], dtype=np.float32)
    wproj = np.asarray(inputs["wproj"], dtype=np.float32)
    pred_w = np.asarray(inputs["pred_w"], dtype=np.float32)

    # this kernel folds away the (identity) LN affine and (zero) biases;
    # verify that assumption against the actual inputs
    assert np.all(np.asarray(inputs["ln1_g"]) == 1), "nonconst ln1_g"
    assert np.all(np.asarray(inputs["ln2_g"]) == 1), "nonconst ln2_g"
    assert np.all(np.asarray(inputs["ln1_b"]) == 0), "nonzero ln1_b"
    assert np.all(np.asarray(inputs["ln2_b"]) == 0), "nonzero ln2_b"
    for bname in ("bqkv", "bo", "bfc", "bproj"):
        assert np.all(np.asarray(inputs[bname]) == 0), f"nonzero {bname}"

    d2 = np.broadcast_to(delim, (B, N, 1, W))
    fx = np.concatenate([x, d2, f, d2], axis=-2).reshape(B, S, W)
    fx = fx + _sinusoidal_pos_emb(S, W)[None]

    nf_idx = (np.arange(N)[:, None] * BLK + (F + np.arange(T + 2))[None, :]).reshape(-1)
    jj = np.arange(NNF) // (T + 2)
    rr = np.arange(NNF) % (T + 2)
    mask = np.full((NNF, N), NEG, np.float32)
    for i in range(N):
        allowed = ((rr <= T) & (jj <= i)) | ((rr == T + 1) & (jj == i - 1))
        mask[allowed, i] = 0.0

    bf = ml_dtypes.bfloat16
    wqkvT = np.ascontiguousarray(wqkv.transpose(0, 2, 1)).astype(bf)
    woT = np.ascontiguousarray(wo.transpose(0, 2, 1)).astype(bf)
    wfcT = np.ascontiguousarray(wfc.transpose(0, 2, 1)).astype(bf)
    wprojT = np.ascontiguousarray(wproj.transpose(0, 2, 1)).astype(bf)
    predT = np.ascontiguousarray(pred_w.T)

    cvec = np.zeros((128, 4), np.float32)
    cvec[:, 0] = 1.0 / W
    cvec[:, 1] = 1.0
    cvec[:, 2] = EPS

    if _PROG is None:
        import os
        _PROG = build_program(ln_pb=os.environ.get("LN_PB", "1") == "1",
                              att_pb=os.environ.get("ATT_PB", "1") == "1",
                              att_pipe=os.environ.get("ATT_PIPE", "1") == "1",
                              psum_rec=os.environ.get("PSUM_REC", "0") == "1")
    nc = _PROG

    in_maps = []
    for c in range(8):
        b, slot = c // 4, c % 4
        cf = CORE_FRAMES[slot]
        fr_idx = np.concatenate([np.arange(i * BLK, i * BLK + F) for i in cf])
        tok = np.concatenate([nf_idx, fr_idx])
        h0T = np.ascontiguousarray(fx[b, tok, :].T)
        in_maps.append({
            "h0": h0T,
            "wqkvT": wqkvT, "woT": woT, "wfcT": wfcT, "wprojT": wprojT,
            "predT": predT,
            "nfmask": np.ascontiguousarray(np.repeat(mask[:, cf], F, axis=1)),
            "cvec": cvec,
        })

    res = run_bass_kernel_spmd(nc, in_maps, list(range(8)))

    out = np.zeros((B, N, F, 1024), np.float32)
    for c in range(8):
        b, slot = c // 4, c % 4
        lo = res.results[c]["logits"].reshape(4, F, 1024)
        for si, i in enumerate(CORE_FRAMES[slot]):
            if slot == 3 and si == 0:
                continue
            out[b, i] = lo[si]
    return out.reshape(B, N * F, 1024)


# revision 25
# speedup vs baseline: 1.0988x; 1.0699x over previous
"""Trainium2 Bass kernel for nn_Decoder_75892072120909 (sparse-attention decoder).

Self-contained: takes FULL inputs (as produced by the problem's setup_inputs),
runs an 8-core SPMD Bass kernel, returns the FULL output [2, 1920, 1024].

Sharding: 2 batches x 4 cores; each core owns 4 frame blocks (the last core of
a batch owns frames [11,12,13,14]; frame 11 is taken from the previous core so
every core runs the identical SPMD program). Each core also replicates the
tiny "non-frame" token trajectory (delim + dynamics tokens, 12 per block = 180
per batch) whose attention is the identity (those tokens attend only to
themselves), so no cross-core communication is needed.

On-core layout: activations are feature-on-partition ("transposed world",
hT [512, Ntok]) so every matmul consumes operands natively. The residual
stream hT stays fp32(r); everything downstream of the LayerNorms (LN outputs,
q/k/v, exp'd scores, gelu outputs and all weights) is bf16, which keeps every
matmul at 1 cycle/row including the N=128 frame-attention blocks (f32r drops
to 4 cycles/row below N=256). Softmax denominators use the single-instruction
DVE reciprocal_approx_fast (~18 bits) instead of the 8-cycle iterative
reciprocal. Non-frame tokens get no q (they are never attention queries) and
in the last layer skip o-proj/LN2/MLP (their logits are discarded).
LayerNorm statistics use ones-vector matmuls on the tensor engine;
rstd = exp(-0.5*ln(var+eps)) keeps the scalar engine on the natural_log_exp
table set shared with the attention exponentials. LN1 of layer l+1 is emitted
inside layer l's MLP tail so its stats/broadcast overlap the down-proj stream.
"""

import sys
import numpy as np

for _p in ("/opt/trn_rl_repo", "/root/.axon_site/_ro/trn_rl_repo"):
    if _p not in sys.path:
        sys.path.append(_p)

import ml_dtypes
import concourse.bass as bass
import concourse.tile as tile
from concourse import mybir
from concourse.bass_utils import run_bass_kernel_spmd

# ---------------- problem constants (hardcoded) ----------------
F = 128           # frame tokens per block
T = 10            # dynamics tokens per block
BLK = F + T + 2   # 140
N = 15            # frame blocks
B = 2
W = 512
L = 8
H = 8
DH = 64
S = N * BLK       # 2100
NNF = N * (T + 2)  # 180 non-frame tokens per batch
NQ = 4 * F        # 512 frame-token queries per core
NTOK = NNF + NQ   # 692 tokens per core
EPS = 1e-5
NEG = -1e30
SCALE = 1.0 / np.sqrt(DH)
CORE_FRAMES = [[0, 1, 2, 3], [4, 5, 6, 7], [8, 9, 10, 11], [11, 12, 13, 14]]
F32 = mybir.dt.float32
F32R = mybir.dt.float32r
BF16 = mybir.dt.bfloat16
AF = mybir.ActivationFunctionType
OP = mybir.AluOpType
TP = ((0, 346), (346, NTOK))        # full token passes
TPL = ((NNF, 436), (436, NTOK))     # last-layer passes (frame tokens only)
# attention-aligned token chunks: 0-1 = the 180 nf tokens, 2-5 = frame tokens
TCH = ((0, 128), (128, 180), (180, 308), (308, 436), (436, 564), (564, 692))


def build_program(waitsplit=True, ln_pb=True, att_pb=True, att_pipe=True, psum_rec=False):
    nc = bass.Bass("TRN2", target_bir_lowering=False, debug=False, num_devices=8)

    h0 = nc.dram_tensor("h0", [W, NTOK], F32, kind="ExternalInput").ap()
    wqkvT_d = nc.dram_tensor("wqkvT", [L, W, 3 * W], BF16, kind="ExternalInput").ap()
    woT_d = nc.dram_tensor("woT", [L, W, W], BF16, kind="ExternalInput").ap()
    wfcT_d = nc.dram_tensor("wfcT", [L, W, 4 * W], BF16, kind="ExternalInput").ap()
    wprojT_d = nc.dram_tensor("wprojT", [L, 4 * W, W], BF16, kind="ExternalInput").ap()
    predT_d = nc.dram_tensor("predT", [W, 1024], F32, kind="ExternalInput").ap()
    mask_d = nc.dram_tensor("nfmask", [NNF, 512], F32, kind="ExternalInput").ap()
    cvec_d = nc.dram_tensor("cvec", [128, 4], F32, kind="ExternalInput").ap()
    out_d = nc.dram_tensor("logits", [NQ, 1024], F32, kind="ExternalOutput").ap()

    with tile.TileContext(nc) as tc:
        _build(tc, h0, wqkvT_d, woT_d, wfcT_d, wprojT_d, predT_d, mask_d,
               cvec_d, out_d, ln_pb, att_pb, att_pipe, psum_rec)

    from concourse.library_overlay import lower_extended_insts
    lower_extended_insts(nc)
    if waitsplit:
        from waitsplit_embedded import split_excess_waits
        split_excess_waits(nc)
    return nc


def _build(tc, h0, wqkvT_d, woT_d, wfcT_d, wprojT_d, predT_d, mask_d,
           cvec_d, out_d, ln_pb=True, att_pb=True, att_pipe=True, psum_rec=True):
    nc = tc.nc
    from contextlib import ExitStack
    ctx = ExitStack()

    def pool(name, bufs, **kw):
        return ctx.enter_context(tc.tile_pool(name=name, bufs=bufs, **kw))

    state = pool("state", 1)
    apool = pool("apool", 3)
    qkp = pool("qkp", 1)
    vp = pool("vp", 1)
    attp = pool("attp", 1)
    ep = pool("ep", 7)
    eop = pool("eop", 4)
    up = pool("up", 3)
    sq = pool("sq", 4)
    sp = pool("sp", 2)
    rp = pool("rp", 2)
    bc = pool("bc", 2)
    wq_p = pool("wq", 1)
    wo_p = pool("wo", 1)
    wfc_p = pool("wfc", 1)
    wpr_p = pool("wpr", 1)
    cst = pool("cst", 1)
    lout = pool("lout", 2)

    # PSUM: 8 banks. psA(mm)x4 rotate through everything transient (LN stats,
    # qkv/v/score/o-proj/up psums); psM g0..g3 are the long-lived accumulators
    # (per-head attention oT during attention, MLP down-proj acc during the
    # MLP). All broadcasts go through gpsimd partition_broadcast into SBUF so
    # the tensor engine never sits behind a scalar/vector chain.
    psA = pool("psA", 4, space="PSUM")
    psM = pool("psM", 1, space="PSUM")

    # ---- constants ----
    ones_inv = cst.tile([128, 1], F32R, name="ones_inv")   # value 1/512
    nc.sync.dma_start(out=ones_inv, in_=cvec_d[:, 0:1].bitcast(F32R))
    cvec_t = cst.tile([128, 4], F32, name="cvec_t")
    nc.sync.dma_start(out=cvec_t, in_=cvec_d)
    ones_row = cst.tile([1, 128], F32R, name="ones_row")
    nc.sync.dma_start(out=ones_row, in_=cvec_d[:, 1:2].rearrange("p o -> o p").bitcast(F32R))
    maskb_t = cst.tile([128, 2, 512], F32, name="maskb_t")
    nc.sync.dma_start(out=maskb_t[0:128, 0, :], in_=mask_d[0:128, :])
    nc.sync.dma_start(out=maskb_t[0:52, 1, :], in_=mask_d[128:180, :])

    # ---- persistent activations ----
    hT = state.tile([128, 4, NTOK], F32R, name="hT")
    nc.sync.dma_start(out=hT, in_=h0.rearrange("(c p) t -> p c t", p=128).bitcast(F32R))
    qkT = qkp.tile([128, 8, NTOK], BF16, name="qkT")
    v_aug = vp.tile([128, 6, H, DH + 1], BF16, name="v_aug")
    for ci in range(6):
        nc.gpsimd.memset(v_aug[:, ci, :, DH:DH + 1], 1.0)
    attnT = attp.tile([128, 4, NTOK], BF16, name="attnT")
    pred_t = cst.tile([128, 4, 1024], F32R, name="pred_t")
    nc.sync.dma_start(out=pred_t,
                      in_=predT_d.rearrange("(c p) f -> p c f", p=128).bitcast(F32R))

    # partition_broadcast lives in the Q7 "proxy" ucode library
    from concourse import library_config
    nc.gpsimd.load_library(library_config.proxy)

    def ln_stats(t0, t1, vt, row):
        """PE stats matmuls + vector mu/var chain for tokens [t0,t1).
        Writes the variance into row `row` of the shared tile `vt` so the
        scalar Ln/Exp of both passes run as ONE pair of instructions (keeps
        the act-table flips at exactly 2 per layer)."""
        n = t1 - t0
        mu_ps = psA.tile([1, 512], F32, tag="mm", name=f"mu{nc.next_id()}")
        ms_ps = psA.tile([1, 512], F32, tag="mm", name=f"ms{nc.next_id()}")
        for c in range(4):
            nc.tensor.matmul(mu_ps[:, 0:n], ones_inv, hT[:, c, t0:t1],
                             start=(c == 0), stop=(c == 3))
        for c in range(4):
            hsq = sq.tile([128, 346], F32R, tag="hsq", name=f"hsq{nc.next_id()}")
            nc.gpsimd.tensor_mul(hsq[:, 0:n], hT[:, c, t0:t1], hT[:, c, t0:t1])
            nc.tensor.matmul(ms_ps[:, 0:n], ones_inv, hsq[:, 0:n],
                             start=(c == 0), stop=(c == 3))
        mu = sp.tile([1, 346], F32R, tag="mu", name=f"muv{nc.next_id()}")
        nc.vector.tensor_copy(out=mu[:, 0:n], in_=mu_ps[:, 0:n])
        # broadcast of mu can start as soon as the copy lands
        mu_b = bc.tile([128, 346], F32R, tag="mu_b", name=f"mub{nc.next_id()}")
        nc.gpsimd.partition_broadcast(mu_b[:, 0:n], mu[:, 0:n])
        musq = sp.tile([1, 346], F32, tag="musq", name=f"mq{nc.next_id()}")
        nc.vector.tensor_mul(musq[:, 0:n], mu[:, 0:n], mu[:, 0:n])
        nc.vector.tensor_tensor(out=vt[0:1, row, 0:n], in0=ms_ps[:, 0:n],
                                in1=musq[:, 0:n], op=OP.subtract)
        return mu_b

    def ln_var_tile():
        # both passes' variances batched along the free dim of one partition
        return sp.tile([1, 2, 346], F32, tag="var", name=f"var{nc.next_id()}")

    def ln_finish1(dst, mu_b, vt, row, t0, t1):
        """per-pass Ln+Exp chain (pipelines against the other pass's matmuls)."""
        n = t1 - t0
        lnv = sp.tile([1, 2, 346], F32, tag="lnv", name=f"lnv{nc.next_id()}")
        nc.scalar.activation(lnv[0:1, row, 0:n], vt[0:1, row, 0:n], AF.Ln,
                             bias=cvec_t[0:1, 2:3])
        rstd = sp.tile([1, 2, 346], F32R, tag="rstd", name=f"rsd{nc.next_id()}")
        nc.scalar.activation(rstd[0:1, row, 0:n], lnv[0:1, row, 0:n],
                             AF.Exp, scale=-0.5)
        rs_b = bc.tile([128, 346], F32R, tag="rs_b", name=f"rsb{nc.next_id()}")
        nc.gpsimd.partition_broadcast(rs_b[:, 0:n], rstd[0:1, row, 0:n])
        for c in range(4):
            nc.vector.tensor_tensor(out=dst[:, c, t0:t1], in0=hT[:, c, t0:t1],
                                    in1=mu_b[:, 0:n], op=OP.subtract)
            nc.vector.tensor_mul(dst[:, c, t0:t1], dst[:, c, t0:t1],
                                 rs_b[:, 0:n])
        return rstd

    def ln_finish(dst, mu_bs, vt, passes):
        """one batched Ln+Exp for both passes, then per-pass bcast + apply."""
        nn = passes[0][1] - passes[0][0]
        assert all(t1 - t0 == nn for t0, t1 in passes)
        lnv = sp.tile([1, 2, 346], F32, tag="lnv", name=f"lnv{nc.next_id()}")
        nc.scalar.activation(lnv[:, :, 0:nn], vt[:, :, 0:nn], AF.Ln,
                             bias=cvec_t[0:1, 2:3])
        rstd = sp.tile([1, 2, 346], F32R, tag="rstd", name=f"rsd{nc.next_id()}")
        nc.scalar.activation(rstd[:, :, 0:nn], lnv[:, :, 0:nn],
                             AF.Exp, scale=-0.5)
        for pi, (t0, t1) in enumerate(passes):
            n = t1 - t0
            rs_b = bc.tile([128, 346], F32R, tag="rs_b", name=f"rsb{nc.next_id()}")
            nc.gpsimd.partition_broadcast(rs_b[:, 0:n], rstd[0:1, pi, 0:n])
            mu_b = mu_bs[pi]
            for c in range(4):
                nc.vector.tensor_tensor(out=dst[:, c, t0:t1], in0=hT[:, c, t0:t1],
                                        in1=mu_b[:, 0:n], op=OP.subtract)
                nc.vector.tensor_mul(dst[:, c, t0:t1], dst[:, c, t0:t1],
                                     rs_b[:, 0:n])
        return rstd

    aT = apool.tile([128, 4, NTOK], BF16, tag="a", name="aT0")
    vt0 = ln_var_tile()
    for pi, (t0, t1) in enumerate(TP):
        ln_finish1(aT, ln_stats(t0, t1, vt0, pi), vt0, pi, t0, t1)

    for l in range(L):
        last = (l == L - 1)
        wq_t = wq_p.tile([128, 4, 3 * W], BF16, tag="wq", name=f"wq{l}")
        nc.sync.dma_start(out=wq_t,
                          in_=wqkvT_d[l].rearrange("(c p) f -> p c f", p=128))
        wo_t = wo_p.tile([128, 4, W], BF16, tag="wo", name=f"wo{l}")
        nc.sync.dma_start(out=wo_t,
                          in_=woT_d[l].rearrange("(c p) f -> p c f", p=128))
        wfc_t = wfc_p.tile([128, 4, 4 * W], BF16, tag="wfc", name=f"wfc{l}")
        nc.sync.dma_start(out=wfc_t,
                          in_=wfcT_d[l].rearrange("(c p) f -> p c f", p=128))
        wpr_t = wpr_p.tile([128, 16, W], BF16, tag="wpr", name=f"wpr{l}")
        nc.sync.dma_start(out=wpr_t,
                          in_=wprojT_d[l].rearrange("(c p) f -> p c f", p=128))

        # ---- k (all tokens) and q (frame tokens only) ----
        for t0, t1 in TP:
            n = t1 - t0
            for fc in range(4, 8):      # k chunks
                ps = psA.tile([128, 512], F32, tag="mm", name=f"k{l}_{fc}_{t0}")
                for c in range(4):
                    nc.tensor.matmul(ps[:, 0:n], wq_t[:, c, 128 * fc:128 * fc + 128],
                                     aT[:, c, t0:t1], start=(c == 0), stop=(c == 3))
                nc.vector.tensor_copy(out=qkT[:, fc, t0:t1], in_=ps[:, 0:n])
            q0 = max(t0, NNF)
            nq = t1 - q0
            for fc in range(4):         # q chunks, frame tokens only
                ps = psA.tile([128, 512], F32, tag="mm", name=f"q{l}_{fc}_{t0}")
                for c in range(4):
                    nc.tensor.matmul(ps[:, 0:nq], wq_t[:, c, 128 * fc:128 * fc + 128],
                                     aT[:, c, q0:t1], start=(c == 0), stop=(c == 3))
                nc.scalar.activation(qkT[:, fc, q0:t1], ps[:, 0:nq], AF.Copy)

        # ---- v (token-on-partition, for PV stationary) ----
        for ci, (t0, t1) in enumerate(TCH):
            rows = t1 - t0
            ps = psA.tile([128, 512], F32, tag="mm", name=f"v{l}_{ci}")
            for c in range(4):
                nc.tensor.matmul(ps[0:rows, :], aT[:, c, t0:t1],
                                 wq_t[:, c, 1024:1536], start=(c == 0), stop=(c == 3))
            nc.vector.tensor_copy(
                out=v_aug[0:rows, ci, :, 0:DH],
                in_=ps[0:rows, :].rearrange("p (hh d) -> p hh d", hh=8))

        # ---- non-frame columns of attnT = v_nf (feature-major matmul) ----
        if not last:
            for fc in range(4):
                ps = psA.tile([128, 512], F32, tag="mm", name=f"vt{l}_{fc}")
                for c in range(4):
                    nc.tensor.matmul(ps[:, 0:NNF],
                                     wq_t[:, c, 1024 + 128 * fc:1152 + 128 * fc],
                                     aT[:, c, 0:NNF], start=(c == 0), stop=(c == 3))
                nc.scalar.activation(attnT[:, fc, 0:NNF], ps[:, 0:NNF], AF.Copy)

        # ---- attention: head-pipelined (PV of head h emitted after the
        # scores of head h+1, so the mask-add/exp chain is covered by PE
        # work and the PE never waits on it) ----
        ebuf = {}

        def att_scores(h):
            r0 = 64 * (h % 2)
            qc, kc = h // 2, 4 + h // 2
            es = []
            for ci, rows in ((0, 128), (1, 52)):
                sc = psA.tile([128, 512], F32, tag="mm", name=f"snf{l}_{h}_{ci}")
                nc.tensor.matmul(sc[0:rows, :],
                                 qkT[r0:r0 + 64, kc, 128 * ci:128 * ci + rows],
                                 qkT[r0:r0 + 64, qc, NNF:NTOK],
                                 start=True, stop=True)
                nc.vector.tensor_add(out=sc[0:rows, :], in0=sc[0:rows, :],
                                     in1=maskb_t[0:rows, ci, :])
                e = ep.tile([128, 512], BF16, tag="e", name=f"e{l}_{h}_{ci}")
                nc.scalar.activation(e[0:rows, :], sc[0:rows, :], AF.Exp,
                                     scale=SCALE)
                es.append(e)
            sc = psA.tile([128, 512], F32, tag="mm", name=f"sow{l}_{h}")
            for a in range(4):
                t0 = NNF + 128 * a
                nc.tensor.matmul(sc[:, 128 * a:128 * a + 128],
                                 qkT[r0:r0 + 64, kc, t0:t0 + 128],
                                 qkT[r0:r0 + 64, qc, t0:t0 + 128],
                                 start=(a == 0), stop=(a == 3))
            e2 = eop.tile([128, 512], BF16, tag="e2", name=f"e2{l}_{h}")
            nc.scalar.activation(e2, sc, AF.Exp, scale=SCALE)
            ebuf[h] = (es, e2)

        def att_pv(h):
            r0 = 64 * (h % 2)
            es, e2 = ebuf.pop(h)
            oT = psM.tile([128, 512], F32, tag=f"g{h % 4}", name=f"oT{l}_{h}")
            oT = oT[0:65, :]
            for ci, rows in ((0, 128), (1, 52)):
                nc.tensor.matmul(oT, v_aug[0:rows, ci, h, :], es[ci][0:rows, :],
                                 start=(ci == 0), stop=False)
            for a in range(4):
                nc.tensor.matmul(oT[:, 128 * a:128 * a + 128],
                                 v_aug[:, 2 + a, h, :],
                                 e2[:, 128 * a:128 * a + 128],
                                 start=False, stop=(a == 3))
            rec = rp.tile([1, 512], F32, tag="rec", name=f"rec{l}_{h}")
            if psum_rec:
                nc.vector.reciprocal_approx_fast(out=rec, in_=oT[64:65, :])
            else:
                den = rp.tile([1, 512], F32, tag="den", name=f"den{l}_{h}")
                nc.vector.tensor_copy(out=den, in_=oT[64:65, :])
                nc.vector.reciprocal_approx_fast(out=rec, in_=den)
            dst = attnT[r0:r0 + 64, h // 2, NNF:NTOK]
            nc.scalar.activation(dst, oT[0:64, :], AF.Copy)
            if att_pb:
                rb = bc.tile([128, 512], F32, tag="rb", name=f"rb{l}_{h}")
                nc.gpsimd.partition_broadcast(rb, rec)
                nc.vector.tensor_mul(dst, dst, rb[r0:r0 + 64, :])
            else:
                rec_r = rp.tile([1, 512], F32R, tag="recr", name=f"recr{l}_{h}")
                with nc.allow_low_precision(reason="f32r broadcast operand"):
                    nc.vector.tensor_copy(out=rec_r, in_=rec)
                rb = psA.tile([128, 512], F32, tag="mm", name=f"rb{l}_{h}")
                nc.tensor.matmul(rb[0:64, :], ones_row[:, 0:64], rec_r,
                                 start=True, stop=True)
                nc.vector.tensor_mul(dst, dst, rb[0:64, :])

        if att_pipe:
            for h in range(H):
                att_scores(h)
                if h > 1:
                    att_pv(h - 2)
            att_pv(H - 2)
            att_pv(H - 1)
        else:
            for h in range(H):
                att_scores(h)
                att_pv(h)

        # ---- O-proj + residual, with LN2 stats interleaved so each pass's
        # stats/chain overlaps the next pass's matmuls ----
        passes = TPL if last else TP
        mT = apool.tile([128, 4, NTOK], BF16, tag="a", name=f"mT{l}")
        vt2 = ln_var_tile()
        mu_bs2 = []
        for pi, (t0, t1) in enumerate(passes):
            n = t1 - t0
            for fc in range(4):
                ps = psA.tile([128, 512], F32, tag="mm", name=f"op{l}_{fc}_{t0}")
                for c in range(4):
                    nc.tensor.matmul(ps[:, 0:n], wo_t[:, c, 128 * fc:128 * fc + 128],
                                     attnT[:, c, t0:t1], start=(c == 0), stop=(c == 3))
                nc.vector.tensor_add(out=hT[:, fc, t0:t1], in0=hT[:, fc, t0:t1],
                                     in1=ps[:, 0:n])
            mu_bs2.append(ln_stats(t0, t1, vt2, pi))
            if pi > 0:
                ln_finish1(mT, mu_bs2[pi - 1], vt2, pi - 1, *passes[pi - 1])
        rstd2 = ln_finish1(mT, mu_bs2[-1], vt2, len(passes) - 1, *passes[-1])
        # preload the gelu table; depends on the last LN2 Exp so the load
        # slots in right after the LN2 scalar work
        dmy = sp.tile([1, 346], F32, tag="dmy", name=f"dmy{l}")
        nc.scalar.activation(dmy[0:1, 0:1], rstd2[0:1, 1, 0:1], AF.Gelu)
        st1 = {}
        vt1 = ln_var_tile() if not last else None
        for pi, (t0, t1) in enumerate(passes):
            n = t1 - t0
            acc = [psM.tile([128, 512], F32, tag=f"g{i}", name=f"acc{l}_{t0}_{i}")
                   for i in range(4)]
            for uc in range(16):
                ups = psA.tile([128, 512], F32, tag="mm", name=f"u{l}_{t0}_{uc}")
                for c in range(4):
                    nc.tensor.matmul(ups[:, 0:n], wfc_t[:, c, 128 * uc:128 * uc + 128],
                                     mT[:, c, t0:t1], start=(c == 0), stop=(c == 3))
                ug = up.tile([128, 346], BF16, tag="ug", name=f"ug{l}_{t0}_{uc}")
                nc.scalar.activation(ug[:, 0:n], ups[:, 0:n], AF.Gelu)
                for fc in range(4):
                    nc.tensor.matmul(acc[fc][:, 0:n],
                                     wpr_t[:, uc, 128 * fc:128 * fc + 128],
                                     ug[:, 0:n], start=(uc == 0), stop=(uc == 15))
            for fc in range(4):
                nc.vector.tensor_add(out=hT[:, fc, t0:t1], in0=hT[:, fc, t0:t1],
                                     in1=acc[fc][:, 0:n])
            # LN1 stats of the next layer for this token pass: the PE matmuls
            # and vector chain run while the other pass's MLP streams.
            if not last:
                st1[pi] = ln_stats(TP[pi][0], TP[pi][1], vt1, pi)
        # switch the table back via Ln (Ln lives only in natural_log_exp, so
        # walrus loads the set that also covers the attention/LN exps);
        # reading the last ug makes the load wait for the final gelu
        dmy2 = sp.tile([1, 346], F32, tag="dmy2", name=f"dmy2{l}")
        nc.scalar.activation(dmy2[0:1, 0:1], ug[0:1, 0:1], AF.Ln, bias=cvec_t[0:1, 1:2])
        if not last:
            aT = apool.tile([128, 4, NTOK], BF16, tag="a", name=f"aT{l + 1}")
            for pi, (t0, t1) in enumerate(TP):
                ln_finish1(aT, st1.pop(pi), vt1, pi, t0, t1)

    # ---- logits for frame tokens ----
    for k in range(4):
        for nb in range(2):
            ps = psA.tile([128, 512], F32, tag="mm", name=f"lg{k}_{nb}")
            for c in range(4):
                nc.tensor.matmul(ps, hT[:, c, NNF + 128 * k:NNF + 128 * k + 128],
                                 pred_t[:, c, 512 * nb:512 * nb + 512],
                                 start=(c == 0), stop=(c == 3))
            lo = lout.tile([128, 512], F32, tag="lo", name=f"lo{k}_{nb}")
            nc.vector.tensor_copy(out=lo, in_=ps)
            nc.sync.dma_start(out=out_d[128 * k:128 * k + 128, 512 * nb:512 * nb + 512],
                              in_=lo)

    ctx.close()


# (waitsplit embedded so kernel.py is self-contained)
import types as _types
waitsplit_embedded = _types.ModuleType("waitsplit_embedded")


def _split_excess_waits(nc):
    n_split = 0
    for fn in nc.m.functions:
        for bb in fn.blocks:
            insts = list(bb.instructions)
            new_list = []
            changed = False
            for inst in insts:
                si = getattr(inst, "sync_info", None)
                waits = list(si.on_wait) if si is not None and si.on_wait else []
                cap = 2 if isinstance(inst, mybir.InstEventSemaphore) else 1
                if len(waits) > cap:
                    changed = True
                    keep = waits[-cap:]
                    for w in waits[:-cap]:
                        n_split += 1
                        nop = mybir.InstNoOp(
                            name=f"WSPLIT-{n_split}-{inst.name}",
                            engine=inst.engine,
                            ins=[], outs=[],
                            sync_info=mybir.SyncInfo(on_wait=[w], on_update=[]),
                        )
                        try:
                            nop.bass_nofuse = True
                        except Exception:
                            pass
                        new_list.append(nop)
                    inst.sync_info = mybir.SyncInfo(on_wait=keep,
                                                    on_update=list(si.on_update))
                new_list.append(inst)
            if changed:
                try:
                    bb.instructions = new_list
                except Exception:
                    bb.instructions.clear()
                    bb.instructions.extend(new_list)
    return n_split


waitsplit_embedded.split_excess_waits = _split_excess_waits
sys.modules["waitsplit_embedded"] = waitsplit_embedded


# ---------------- host side ----------------

def _sinusoidal_pos_emb(n_pos, d, n=10000.0):
    pos = np.arange(n_pos, dtype=np.float32)[:, None]
    den = np.power(n, 2.0 * np.arange(d // 2, dtype=np.float32) / d).astype(np.float32)
    emb = np.zeros((n_pos, d), dtype=np.float32)
    emb[:, 0::2] = np.sin(pos / den)
    emb[:, 1::2] = np.cos(pos / den)
    return emb


_PROG = None


def kernel(**inputs):
    global _PROG
    x = np.ascontiguousarray(np.asarray(inputs["x"], dtype=np.float32))
    f = np.ascontiguousarray(np.asarray(inputs["f"], dtype=np.float32))
    delim = np.asarray(inputs["frame_delim"], dtype=np.float32)
    wqkv = np.asarray(inputs["wqkv"], dtype=np.float32)
    wo = np.asarray(inputs["wo"], dtype=np.float32)
    wfc = np.asarray(inputs["wfc"], dtype=np.float32)
    wproj = np.asarray(inputs["wproj"], dtype=np.float32)
    pred_w = np.asarray(inputs["pred_w"], dtype=np.float32)

    # this kernel folds away the (identity) LN affine and (zero) biases;
    # verify that assumption against the actual inputs
    assert np.all(np.asarray(inputs["ln1_g"]) == 1), "nonconst ln1_g"
    assert np.all(np.asarray(inputs["ln2_g"]) == 1), "nonconst ln2_g"
    assert np.all(np.asarray(inputs["ln1_b"]) == 0), "nonzero ln1_b"
    assert np.all(np.asarray(inputs["ln2_b"]) == 0), "nonzero ln2_b"
    for bname in ("bqkv", "bo", "bfc", "bproj"):
        assert np.all(np.asarray(inputs[bname]) == 0), f"nonzero {bname}"

    d2 = np.broadcast_to(delim, (B, N, 1, W))
    fx = np.concatenate([x, d2, f, d2], axis=-2).reshape(B, S, W)
    fx = fx + _sinusoidal_pos_emb(S, W)[None]

    nf_idx = (np.arange(N)[:, None] * BLK + (F + np.arange(T + 2))[None, :]).reshape(-1)
    jj = np.arange(NNF) // (T + 2)
    rr = np.arange(NNF) % (T + 2)
    mask = np.full((NNF, N), NEG, np.float32)
    for i in range(N):
        allowed = ((rr <= T) & (jj <= i)) | ((rr == T + 1) & (jj == i - 1))
        mask[allowed, i] = 0.0

    bf = ml_dtypes.bfloat16
    wqkvT = np.ascontiguousarray(wqkv.transpose(0, 2, 1)).astype(bf)
    woT = np.ascontiguousarray(wo.transpose(0, 2, 1)).astype(bf)
    wfcT = np.ascontiguousarray(wfc.transpose(0, 2, 1)).astype(bf)
    wprojT = np.ascontiguousarray(wproj.transpose(0, 2, 1)).astype(bf)
    predT = np.ascontiguousarray(pred_w.T)

    cvec = np.zeros((128, 4), np.float32)
    cvec[:, 0] = 1.0 / W
    cvec[:, 1] = 1.0
    cvec[:, 2] = EPS

    if _PROG is None:
        import os
        _PROG = build_program(ln_pb=os.environ.get("LN_PB", "1") == "1",
                              att_pb=os.environ.get("ATT_PB", "1") == "1",
                              att_pipe=os.environ.get("ATT_PIPE", "1") == "1",
                              psum_rec=os.environ.get("PSUM_REC", "0") == "1")
    nc = _PROG

    in_maps = []
    for c in range(8):
        b, slot = c // 4, c % 4
        cf = CORE_FRAMES[slot]
        fr_idx = np.concatenate([np.arange(i * BLK, i * BLK + F) for i in cf])
        tok = np.concatenate([nf_idx, fr_idx])
        h0T = np.ascontiguousarray(fx[b, tok, :].T)
        in_maps.append({
            "h0": h0T,
            "wqkvT": wqkvT, "woT": woT, "wfcT": wfcT, "wprojT": wprojT,
            "predT": predT,
            "nfmask": np.ascontiguousarray(np.repeat(mask[:, cf], F, axis=1)),
            "cvec": cvec,
        })

    res = run_bass_kernel_spmd(nc, in_maps, list(range(8)))

    out = np.zeros((B, N, F, 1024), np.float32)
    for c in range(8):
        b, slot = c // 4, c % 4
        lo = res.results[c]["logits"].reshape(4, F, 1024)
        for si, i in enumerate(CORE_FRAMES[slot]):
            if slot == 3 and si == 0:
                continue
            out[b, i] = lo[si]
    return out.reshape(B, N * F, 1024)


# revision 26
# speedup vs baseline: 1.1020x; 1.0029x over previous
"""Trainium2 Bass kernel for nn_Decoder_75892072120909 (sparse-attention decoder).

Self-contained: takes FULL inputs (as produced by the problem's setup_inputs),
runs an 8-core SPMD Bass kernel, returns the FULL output [2, 1920, 1024].

Sharding: 2 batches x 4 cores; each core owns 4 frame blocks (the last core of
a batch owns frames [11,12,13,14]; frame 11 is taken from the previous core so
every core runs the identical SPMD program). Each core also replicates the
tiny "non-frame" token trajectory (delim + dynamics tokens, 12 per block = 180
per batch) whose attention is the identity (those tokens attend only to
themselves), so no cross-core communication is needed.

On-core layout: activations are feature-on-partition ("transposed world",
hT [512, Ntok]) so every matmul consumes operands natively. The residual
stream hT stays fp32(r); everything downstream of the LayerNorms (LN outputs,
q/k/v, exp'd scores, gelu outputs and all weights) is bf16, which keeps every
matmul at 1 cycle/row including the N=128 frame-attention blocks (f32r drops
to 4 cycles/row below N=256). Softmax denominators use the single-instruction
DVE reciprocal_approx_fast (~18 bits) instead of the 8-cycle iterative
reciprocal. Non-frame tokens get no q (they are never attention queries) and
in the last layer skip o-proj/LN2/MLP (their logits are discarded).
LayerNorm statistics use ones-vector matmuls on the tensor engine;
rstd = exp(-0.5*ln(var+eps)) keeps the scalar engine on the natural_log_exp
table set shared with the attention exponentials. LN1 of layer l+1 is emitted
inside layer l's MLP tail so its stats/broadcast overlap the down-proj stream.
"""

import sys
import numpy as np

for _p in ("/opt/trn_rl_repo", "/root/.axon_site/_ro/trn_rl_repo"):
    if _p not in sys.path:
        sys.path.append(_p)

import ml_dtypes
import concourse.bass as bass
import concourse.tile as tile
from concourse import mybir
from concourse.bass_utils import run_bass_kernel_spmd

# ---------------- problem constants (hardcoded) ----------------
F = 128           # frame tokens per block
T = 10            # dynamics tokens per block
BLK = F + T + 2   # 140
N = 15            # frame blocks
B = 2
W = 512
L = 8
H = 8
DH = 64
S = N * BLK       # 2100
NNF = N * (T + 2)  # 180 non-frame tokens per batch
NQ = 4 * F        # 512 frame-token queries per core
NTOK = NNF + NQ   # 692 tokens per core
EPS = 1e-5
NEG = -1e30
SCALE = 1.0 / np.sqrt(DH)
CORE_FRAMES = [[0, 1, 2, 3], [4, 5, 6, 7], [8, 9, 10, 11], [11, 12, 13, 14]]
F32 = mybir.dt.float32
F32R = mybir.dt.float32r
BF16 = mybir.dt.bfloat16
AF = mybir.ActivationFunctionType
OP = mybir.AluOpType
TP = ((0, 346), (346, NTOK))        # full token passes
TPL = ((NNF, 436), (436, NTOK))     # last-layer passes (frame tokens only)
# attention-aligned token chunks: 0-1 = the 180 nf tokens, 2-5 = frame tokens
TCH = ((0, 128), (128, 180), (180, 308), (308, 436), (436, 564), (564, 692))


def build_program(waitsplit=True, ln_pb=True, att_pb=True, att_pipe=True, psum_rec=False):
    nc = bass.Bass("TRN2", target_bir_lowering=False, debug=False, num_devices=8)

    h0 = nc.dram_tensor("h0", [W, NTOK], F32, kind="ExternalInput").ap()
    wqkvT_d = nc.dram_tensor("wqkvT", [L, W, 3 * W], BF16, kind="ExternalInput").ap()
    woT_d = nc.dram_tensor("woT", [L, W, W], BF16, kind="ExternalInput").ap()
    wfcT_d = nc.dram_tensor("wfcT", [L, W, 4 * W], BF16, kind="ExternalInput").ap()
    wprojT_d = nc.dram_tensor("wprojT", [L, 4 * W, W], BF16, kind="ExternalInput").ap()
    predT_d = nc.dram_tensor("predT", [W, 1024], F32, kind="ExternalInput").ap()
    mask_d = nc.dram_tensor("nfmask", [NNF, 512], F32, kind="ExternalInput").ap()
    cvec_d = nc.dram_tensor("cvec", [128, 4], F32, kind="ExternalInput").ap()
    out_d = nc.dram_tensor("logits", [NQ, 1024], F32, kind="ExternalOutput").ap()

    with tile.TileContext(nc) as tc:
        _build(tc, h0, wqkvT_d, woT_d, wfcT_d, wprojT_d, predT_d, mask_d,
               cvec_d, out_d, ln_pb, att_pb, att_pipe, psum_rec)

    from concourse.library_overlay import lower_extended_insts
    lower_extended_insts(nc)
    if waitsplit:
        from waitsplit_embedded import split_excess_waits
        split_excess_waits(nc)
    return nc


def _build(tc, h0, wqkvT_d, woT_d, wfcT_d, wprojT_d, predT_d, mask_d,
           cvec_d, out_d, ln_pb=True, att_pb=True, att_pipe=True, psum_rec=True):
    nc = tc.nc
    from contextlib import ExitStack
    ctx = ExitStack()

    def pool(name, bufs, **kw):
        return ctx.enter_context(tc.tile_pool(name=name, bufs=bufs, **kw))

    state = pool("state", 1)
    apool = pool("apool", 3)
    qkp = pool("qkp", 1)
    vp = pool("vp", 1)
    attp = pool("attp", 1)
    ep = pool("ep", 7)
    eop = pool("eop", 4)
    up = pool("up", 6)
    sq = pool("sq", 6)
    sp = pool("sp", 2)
    rp = pool("rp", 2)
    bc = pool("bc", 2)
    wq_p = pool("wq", 1)
    wo_p = pool("wo", 1)
    wfc_p = pool("wfc", 1)
    wpr_p = pool("wpr", 1)
    cst = pool("cst", 1)
    lout = pool("lout", 2)

    # PSUM: 8 banks. psA(mm)x4 rotate through everything transient (LN stats,
    # qkv/v/score/o-proj/up psums); psM g0..g3 are the long-lived accumulators
    # (per-head attention oT during attention, MLP down-proj acc during the
    # MLP). All broadcasts go through gpsimd partition_broadcast into SBUF so
    # the tensor engine never sits behind a scalar/vector chain.
    psA = pool("psA", 4, space="PSUM")
    psM = pool("psM", 1, space="PSUM")

    # ---- constants ----
    ones_inv = cst.tile([128, 1], F32R, name="ones_inv")   # value 1/512
    nc.sync.dma_start(out=ones_inv, in_=cvec_d[:, 0:1].bitcast(F32R))
    cvec_t = cst.tile([128, 4], F32, name="cvec_t")
    nc.sync.dma_start(out=cvec_t, in_=cvec_d)
    ones_row = cst.tile([1, 128], F32R, name="ones_row")
    nc.sync.dma_start(out=ones_row, in_=cvec_d[:, 1:2].rearrange("p o -> o p").bitcast(F32R))
    maskb_t = cst.tile([128, 2, 512], F32, name="maskb_t")
    nc.sync.dma_start(out=maskb_t[0:128, 0, :], in_=mask_d[0:128, :])
    nc.sync.dma_start(out=maskb_t[0:52, 1, :], in_=mask_d[128:180, :])

    # ---- persistent activations ----
    hT = state.tile([128, 4, NTOK], F32R, name="hT")
    nc.sync.dma_start(out=hT, in_=h0.rearrange("(c p) t -> p c t", p=128).bitcast(F32R))
    qkT = qkp.tile([128, 8, NTOK], BF16, name="qkT")
    v_aug = vp.tile([128, 6, H, DH + 1], BF16, name="v_aug")
    for ci in range(6):
        nc.gpsimd.memset(v_aug[:, ci, :, DH:DH + 1], 1.0)
    attnT = attp.tile([128, 4, NTOK], BF16, name="attnT")
    pred_t = cst.tile([128, 4, 1024], F32R, name="pred_t")
    nc.sync.dma_start(out=pred_t,
                      in_=predT_d.rearrange("(c p) f -> p c f", p=128).bitcast(F32R))

    # partition_broadcast lives in the Q7 "proxy" ucode library
    from concourse import library_config
    nc.gpsimd.load_library(library_config.proxy)

    def ln_stats(t0, t1, vt, row):
        """PE stats matmuls + vector mu/var chain for tokens [t0,t1).
        Writes the variance into row `row` of the shared tile `vt` so the
        scalar Ln/Exp of both passes run as ONE pair of instructions (keeps
        the act-table flips at exactly 2 per layer)."""
        n = t1 - t0
        mu_ps = psA.tile([1, 512], F32, tag="mm", name=f"mu{nc.next_id()}")
        ms_ps = psA.tile([1, 512], F32, tag="mm", name=f"ms{nc.next_id()}")
        for c in range(4):
            nc.tensor.matmul(mu_ps[:, 0:n], ones_inv, hT[:, c, t0:t1],
                             start=(c == 0), stop=(c == 3))
        for c in range(4):
            hsq = sq.tile([128, 346], F32R, tag="hsq", name=f"hsq{nc.next_id()}")
            nc.gpsimd.tensor_mul(hsq[:, 0:n], hT[:, c, t0:t1], hT[:, c, t0:t1])
            nc.tensor.matmul(ms_ps[:, 0:n], ones_inv, hsq[:, 0:n],
                             start=(c == 0), stop=(c == 3))
        mu = sp.tile([1, 346], F32R, tag="mu", name=f"muv{nc.next_id()}")
        nc.vector.tensor_copy(out=mu[:, 0:n], in_=mu_ps[:, 0:n])
        # broadcast of mu can start as soon as the copy lands
        mu_b = bc.tile([128, 346], F32R, tag="mu_b", name=f"mub{nc.next_id()}")
        nc.gpsimd.partition_broadcast(mu_b[:, 0:n], mu[:, 0:n])
        musq = sp.tile([1, 346], F32, tag="musq", name=f"mq{nc.next_id()}")
        nc.vector.tensor_mul(musq[:, 0:n], mu[:, 0:n], mu[:, 0:n])
        nc.vector.tensor_tensor(out=vt[0:1, row, 0:n], in0=ms_ps[:, 0:n],
                                in1=musq[:, 0:n], op=OP.subtract)
        return mu_b

    def ln_var_tile():
        # both passes' variances batched along the free dim of one partition
        return sp.tile([1, 2, 346], F32, tag="var", name=f"var{nc.next_id()}")

    def ln_finish1(dst, mu_b, vt, row, t0, t1):
        """per-pass Ln+Exp chain (pipelines against the other pass's matmuls)."""
        n = t1 - t0
        lnv = sp.tile([1, 2, 346], F32, tag="lnv", name=f"lnv{nc.next_id()}")
        nc.scalar.activation(lnv[0:1, row, 0:n], vt[0:1, row, 0:n], AF.Ln,
                             bias=cvec_t[0:1, 2:3])
        rstd = sp.tile([1, 2, 346], F32R, tag="rstd", name=f"rsd{nc.next_id()}")
        nc.scalar.activation(rstd[0:1, row, 0:n], lnv[0:1, row, 0:n],
                             AF.Exp, scale=-0.5)
        rs_b = bc.tile([128, 346], F32R, tag="rs_b", name=f"rsb{nc.next_id()}")
        nc.gpsimd.partition_broadcast(rs_b[:, 0:n], rstd[0:1, row, 0:n])
        for c in range(4):
            nc.vector.tensor_tensor(out=dst[:, c, t0:t1], in0=hT[:, c, t0:t1],
                                    in1=mu_b[:, 0:n], op=OP.subtract)
            nc.vector.tensor_mul(dst[:, c, t0:t1], dst[:, c, t0:t1],
                                 rs_b[:, 0:n])
        return rstd

    def ln_finish(dst, mu_bs, vt, passes):
        """one batched Ln+Exp for both passes, then per-pass bcast + apply."""
        nn = passes[0][1] - passes[0][0]
        assert all(t1 - t0 == nn for t0, t1 in passes)
        lnv = sp.tile([1, 2, 346], F32, tag="lnv", name=f"lnv{nc.next_id()}")
        nc.scalar.activation(lnv[:, :, 0:nn], vt[:, :, 0:nn], AF.Ln,
                             bias=cvec_t[0:1, 2:3])
        rstd = sp.tile([1, 2, 346], F32R, tag="rstd", name=f"rsd{nc.next_id()}")
        nc.scalar.activation(rstd[:, :, 0:nn], lnv[:, :, 0:nn],
                             AF.Exp, scale=-0.5)
        for pi, (t0, t1) in enumerate(passes):
            n = t1 - t0
            rs_b = bc.tile([128, 346], F32R, tag="rs_b", name=f"rsb{nc.next_id()}")
            nc.gpsimd.partition_broadcast(rs_b[:, 0:n], rstd[0:1, pi, 0:n])
            mu_b = mu_bs[pi]
            for c in range(4):
                nc.vector.tensor_tensor(out=dst[:, c, t0:t1], in0=hT[:, c, t0:t1],
                                        in1=mu_b[:, 0:n], op=OP.subtract)
                nc.vector.tensor_mul(dst[:, c, t0:t1], dst[:, c, t0:t1],
                                     rs_b[:, 0:n])
        return rstd

    aT = apool.tile([128, 4, NTOK], BF16, tag="a", name="aT0")
    vt0 = ln_var_tile()
    for pi, (t0, t1) in enumerate(TP):
        ln_finish1(aT, ln_stats(t0, t1, vt0, pi), vt0, pi, t0, t1)

    for l in range(L):
        last = (l == L - 1)
        wq_t = wq_p.tile([128, 4, 3 * W], BF16, tag="wq", name=f"wq{l}")
        nc.sync.dma_start(out=wq_t,
                          in_=wqkvT_d[l].rearrange("(c p) f -> p c f", p=128))
        wo_t = wo_p.tile([128, 4, W], BF16, tag="wo", name=f"wo{l}")
        nc.sync.dma_start(out=wo_t,
                          in_=woT_d[l].rearrange("(c p) f -> p c f", p=128))
        wfc_t = wfc_p.tile([128, 4, 4 * W], BF16, tag="wfc", name=f"wfc{l}")
        nc.sync.dma_start(out=wfc_t,
                          in_=wfcT_d[l].rearrange("(c p) f -> p c f", p=128))
        wpr_t = wpr_p.tile([128, 16, W], BF16, tag="wpr", name=f"wpr{l}")
        nc.sync.dma_start(out=wpr_t,
                          in_=wprojT_d[l].rearrange("(c p) f -> p c f", p=128))

        # ---- k (all tokens) and q (frame tokens only) ----
        for t0, t1 in TP:
            n = t1 - t0
            for fc in range(4, 8):      # k chunks
                ps = psA.tile([128, 512], F32, tag="mm", name=f"k{l}_{fc}_{t0}")
                for c in range(4):
                    nc.tensor.matmul(ps[:, 0:n], wq_t[:, c, 128 * fc:128 * fc + 128],
                                     aT[:, c, t0:t1], start=(c == 0), stop=(c == 3))
                nc.vector.tensor_copy(out=qkT[:, fc, t0:t1], in_=ps[:, 0:n])
            q0 = max(t0, NNF)
            nq = t1 - q0
            for fc in range(4):         # q chunks, frame tokens only
                ps = psA.tile([128, 512], F32, tag="mm", name=f"q{l}_{fc}_{t0}")
                for c in range(4):
                    nc.tensor.matmul(ps[:, 0:nq], wq_t[:, c, 128 * fc:128 * fc + 128],
                                     aT[:, c, q0:t1], start=(c == 0), stop=(c == 3))
                nc.scalar.activation(qkT[:, fc, q0:t1], ps[:, 0:nq], AF.Copy)

        # ---- v (token-on-partition, for PV stationary) ----
        for ci, (t0, t1) in enumerate(TCH):
            rows = t1 - t0
            ps = psA.tile([128, 512], F32, tag="mm", name=f"v{l}_{ci}")
            for c in range(4):
                nc.tensor.matmul(ps[0:rows, :], aT[:, c, t0:t1],
                                 wq_t[:, c, 1024:1536], start=(c == 0), stop=(c == 3))
            nc.vector.tensor_copy(
                out=v_aug[0:rows, ci, :, 0:DH],
                in_=ps[0:rows, :].rearrange("p (hh d) -> p hh d", hh=8))

        # ---- non-frame columns of attnT = v_nf (feature-major matmul) ----
        if not last:
            for fc in range(4):
                ps = psA.tile([128, 512], F32, tag="mm", name=f"vt{l}_{fc}")
                for c in range(4):
                    nc.tensor.matmul(ps[:, 0:NNF],
                                     wq_t[:, c, 1024 + 128 * fc:1152 + 128 * fc],
                                     aT[:, c, 0:NNF], start=(c == 0), stop=(c == 3))
                nc.scalar.activation(attnT[:, fc, 0:NNF], ps[:, 0:NNF], AF.Copy)

        # ---- attention: head-pipelined (PV of head h emitted after the
        # scores of head h+1, so the mask-add/exp chain is covered by PE
        # work and the PE never waits on it) ----
        ebuf = {}

        def att_scores(h):
            r0 = 64 * (h % 2)
            qc, kc = h // 2, 4 + h // 2
            es = []
            for ci, rows in ((0, 128), (1, 52)):
                sc = psA.tile([128, 512], F32, tag="mm", name=f"snf{l}_{h}_{ci}")
                nc.tensor.matmul(sc[0:rows, :],
                                 qkT[r0:r0 + 64, kc, 128 * ci:128 * ci + rows],
                                 qkT[r0:r0 + 64, qc, NNF:NTOK],
                                 start=True, stop=True)
                nc.vector.tensor_add(out=sc[0:rows, :], in0=sc[0:rows, :],
                                     in1=maskb_t[0:rows, ci, :])
                e = ep.tile([128, 512], BF16, tag="e", name=f"e{l}_{h}_{ci}")
                nc.scalar.activation(e[0:rows, :], sc[0:rows, :], AF.Exp,
                                     scale=SCALE)
                es.append(e)
            sc = psA.tile([128, 512], F32, tag="mm", name=f"sow{l}_{h}")
            for a in range(4):
                t0 = NNF + 128 * a
                nc.tensor.matmul(sc[:, 128 * a:128 * a + 128],
                                 qkT[r0:r0 + 64, kc, t0:t0 + 128],
                                 qkT[r0:r0 + 64, qc, t0:t0 + 128],
                                 start=(a == 0), stop=(a == 3))
            e2 = eop.tile([128, 512], BF16, tag="e2", name=f"e2{l}_{h}")
            nc.scalar.activation(e2, sc, AF.Exp, scale=SCALE)
            ebuf[h] = (es, e2)

        def att_pv(h):
            r0 = 64 * (h % 2)
            es, e2 = ebuf.pop(h)
            oT = psM.tile([128, 512], F32, tag=f"g{h % 4}", name=f"oT{l}_{h}")
            oT = oT[0:65, :]
            for ci, rows in ((0, 128), (1, 52)):
                nc.tensor.matmul(oT, v_aug[0:rows, ci, h, :], es[ci][0:rows, :],
                                 start=(ci == 0), stop=False)
            for a in range(4):
                nc.tensor.matmul(oT[:, 128 * a:128 * a + 128],
                                 v_aug[:, 2 + a, h, :],
                                 e2[:, 128 * a:128 * a + 128],
                                 start=False, stop=(a == 3))
            rec = rp.tile([1, 512], F32, tag="rec", name=f"rec{l}_{h}")
            if psum_rec:
                nc.vector.reciprocal_approx_fast(out=rec, in_=oT[64:65, :])
            else:
                den = rp.tile([1, 512], F32, tag="den", name=f"den{l}_{h}")
                nc.vector.tensor_copy(out=den, in_=oT[64:65, :])
                nc.vector.reciprocal_approx_fast(out=rec, in_=den)
            dst = attnT[r0:r0 + 64, h // 2, NNF:NTOK]
            nc.scalar.activation(dst, oT[0:64, :], AF.Copy)
            if att_pb:
                rb = bc.tile([128, 512], F32, tag="rb", name=f"rb{l}_{h}")
                nc.gpsimd.partition_broadcast(rb, rec)
                nc.vector.tensor_mul(dst, dst, rb[r0:r0 + 64, :])
            else:
                rec_r = rp.tile([1, 512], F32R, tag="recr", name=f"recr{l}_{h}")
                with nc.allow_low_precision(reason="f32r broadcast operand"):
                    nc.vector.tensor_copy(out=rec_r, in_=rec)
                rb = psA.tile([128, 512], F32, tag="mm", name=f"rb{l}_{h}")
                nc.tensor.matmul(rb[0:64, :], ones_row[:, 0:64], rec_r,
                                 start=True, stop=True)
                nc.vector.tensor_mul(dst, dst, rb[0:64, :])

        if att_pipe:
            for h in range(H):
                att_scores(h)
                if h > 1:
                    att_pv(h - 2)
            att_pv(H - 2)
            att_pv(H - 1)
        else:
            for h in range(H):
                att_scores(h)
                att_pv(h)

        # ---- O-proj + residual, with LN2 stats interleaved so each pass's
        # stats/chain overlaps the next pass's matmuls ----
        passes = TPL if last else TP
        mT = apool.tile([128, 4, NTOK], BF16, tag="a", name=f"mT{l}")
        vt2 = ln_var_tile()
        mu_bs2 = []
        for pi, (t0, t1) in enumerate(passes):
            n = t1 - t0
            for fc in range(4):
                ps = psA.tile([128, 512], F32, tag="mm", name=f"op{l}_{fc}_{t0}")
                for c in range(4):
                    nc.tensor.matmul(ps[:, 0:n], wo_t[:, c, 128 * fc:128 * fc + 128],
                                     attnT[:, c, t0:t1], start=(c == 0), stop=(c == 3))
                nc.vector.tensor_add(out=hT[:, fc, t0:t1], in0=hT[:, fc, t0:t1],
                                     in1=ps[:, 0:n])
            mu_bs2.append(ln_stats(t0, t1, vt2, pi))
            if pi > 0:
                ln_finish1(mT, mu_bs2[pi - 1], vt2, pi - 1, *passes[pi - 1])
        rstd2 = ln_finish1(mT, mu_bs2[-1], vt2, len(passes) - 1, *passes[-1])
        # preload the gelu table; depends on the last LN2 Exp so the load
        # slots in right after the LN2 scalar work
        dmy = sp.tile([1, 346], F32, tag="dmy", name=f"dmy{l}")
        nc.scalar.activation(dmy[0:1, 0:1], rstd2[0:1, 1, 0:1], AF.Gelu)
        st1 = {}
        vt1 = ln_var_tile() if not last else None
        for pi, (t0, t1) in enumerate(passes):
            n = t1 - t0
            acc = [psM.tile([128, 512], F32, tag=f"g{i}", name=f"acc{l}_{t0}_{i}")
                   for i in range(4)]
            for uc in range(16):
                ups = psA.tile([128, 512], F32, tag="mm", name=f"u{l}_{t0}_{uc}")
                for c in range(4):
                    nc.tensor.matmul(ups[:, 0:n], wfc_t[:, c, 128 * uc:128 * uc + 128],
                                     mT[:, c, t0:t1], start=(c == 0), stop=(c == 3))
                ug = up.tile([128, 346], BF16, tag="ug", name=f"ug{l}_{t0}_{uc}")
                nc.scalar.activation(ug[:, 0:n], ups[:, 0:n], AF.Gelu)
                for fc in range(4):
                    nc.tensor.matmul(acc[fc][:, 0:n],
                                     wpr_t[:, uc, 128 * fc:128 * fc + 128],
                                     ug[:, 0:n], start=(uc == 0), stop=(uc == 15))
            for fc in range(4):
                nc.vector.tensor_add(out=hT[:, fc, t0:t1], in0=hT[:, fc, t0:t1],
                                     in1=acc[fc][:, 0:n])
            # LN1 stats of the next layer for this token pass: the PE matmuls
            # and vector chain run while the other pass's MLP streams.
            if not last:
                st1[pi] = ln_stats(TP[pi][0], TP[pi][1], vt1, pi)
        # switch the table back via Ln (Ln lives only in natural_log_exp, so
        # walrus loads the set that also covers the attention/LN exps);
        # reading the last ug makes the load wait for the final gelu
        dmy2 = sp.tile([1, 346], F32, tag="dmy2", name=f"dmy2{l}")
        nc.scalar.activation(dmy2[0:1, 0:1], ug[0:1, 0:1], AF.Ln, bias=cvec_t[0:1, 1:2])
        if not last:
            aT = apool.tile([128, 4, NTOK], BF16, tag="a", name=f"aT{l + 1}")
            for pi, (t0, t1) in enumerate(TP):
                ln_finish1(aT, st1.pop(pi), vt1, pi, t0, t1)

    # ---- logits for frame tokens ----
    for k in range(4):
        for nb in range(2):
            ps = psA.tile([128, 512], F32, tag="mm", name=f"lg{k}_{nb}")
            for c in range(4):
                nc.tensor.matmul(ps, hT[:, c, NNF + 128 * k:NNF + 128 * k + 128],
                                 pred_t[:, c, 512 * nb:512 * nb + 512],
                                 start=(c == 0), stop=(c == 3))
            lo = lout.tile([128, 512], F32, tag="lo", name=f"lo{k}_{nb}")
            nc.vector.tensor_copy(out=lo, in_=ps)
            nc.sync.dma_start(out=out_d[128 * k:128 * k + 128, 512 * nb:512 * nb + 512],
                              in_=lo)

    ctx.close()


# (waitsplit embedded so kernel.py is self-contained)
import types as _types
waitsplit_embedded = _types.ModuleType("waitsplit_embedded")


def _split_excess_waits(nc):
    n_split = 0
    for fn in nc.m.functions:
        for bb in fn.blocks:
            insts = list(bb.instructions)
            new_list = []
            changed = False
            for inst in insts:
                si = getattr(inst, "sync_info", None)
                waits = list(si.on_wait) if si is not None and si.on_wait else []
                cap = 2 if isinstance(inst, mybir.InstEventSemaphore) else 1
                if len(waits) > cap:
                    changed = True
                    keep = waits[-cap:]
                    for w in waits[:-cap]:
                        n_split += 1
                        nop = mybir.InstNoOp(
                            name=f"WSPLIT-{n_split}-{inst.name}",
                            engine=inst.engine,
                            ins=[], outs=[],
                            sync_info=mybir.SyncInfo(on_wait=[w], on_update=[]),
                        )
                        try:
                            nop.bass_nofuse = True
                        except Exception:
                            pass
                        new_list.append(nop)
                    inst.sync_info = mybir.SyncInfo(on_wait=keep,
                                                    on_update=list(si.on_update))
                new_list.append(inst)
            if changed:
                try:
                    bb.instructions = new_list
                except Exception:
                    bb.instructions.clear()
                    bb.instructions.extend(new_list)
    return n_split


waitsplit_embedded.split_excess_waits = _split_excess_waits
sys.modules["waitsplit_embedded"] = waitsplit_embedded


# ---------------- host side ----------------

def _sinusoidal_pos_emb(n_pos, d, n=10000.0):
    pos = np.arange(n_pos, dtype=np.float32)[:, None]
    den = np.power(n, 2.0 * np.arange(d // 2, dtype=np.float32) / d).astype(np.float32)
    emb = np.zeros((n_pos, d), dtype=np.float32)
    emb[:, 0::2] = np.sin(pos / den)
    emb[:, 1::2] = np.cos(pos / den)
    return emb


_PROG = None


def kernel(**inputs):
    global _PROG
    x = np.ascontiguousarray(np.asarray(inputs["x"], dtype=np.float32))
    f = np.ascontiguousarray(np.asarray(inputs["f"], dtype=np.float32))
    delim = np.asarray(inputs["frame_delim"], dtype=np.float32)
    wqkv = np.asarray(inputs["wqkv"], dtype=np.float32)
    wo = np.asarray(inputs["wo"], dtype=np.float32)
    wfc = np.asarray(inputs["wfc"], dtype=np.float32)
    wproj = np.asarray(inputs["wproj"], dtype=np.float32)
    pred_w = np.asarray(inputs["pred_w"], dtype=np.float32)

    # this kernel folds away the (identity) LN affine and (zero) biases;
    # verify that assumption against the actual inputs
    assert np.all(np.asarray(inputs["ln1_g"]) == 1), "nonconst ln1_g"
    assert np.all(np.asarray(inputs["ln2_g"]) == 1), "nonconst ln2_g"
    assert np.all(np.asarray(inputs["ln1_b"]) == 0), "nonzero ln1_b"
    assert np.all(np.asarray(inputs["ln2_b"]) == 0), "nonzero ln2_b"
    for bname in ("bqkv", "bo", "bfc", "bproj"):
        assert np.all(np.asarray(inputs[bname]) == 0), f"nonzero {bname}"

    d2 = np.broadcast_to(delim, (B, N, 1, W))
    fx = np.concatenate([x, d2, f, d2], axis=-2).reshape(B, S, W)
    fx = fx + _sinusoidal_pos_emb(S, W)[None]

    nf_idx = (np.arange(N)[:, None] * BLK + (F + np.arange(T + 2))[None, :]).reshape(-1)
    jj = np.arange(NNF) // (T + 2)
    rr = np.arange(NNF) % (T + 2)
    mask = np.full((NNF, N), NEG, np.float32)
    for i in range(N):
        allowed = ((rr <= T) & (jj <= i)) | ((rr == T + 1) & (jj == i - 1))
        mask[allowed, i] = 0.0

    bf = ml_dtypes.bfloat16
    wqkvT = np.ascontiguousarray(wqkv.transpose(0, 2, 1)).astype(bf)
    woT = np.ascontiguousarray(wo.transpose(0, 2, 1)).astype(bf)
    wfcT = np.ascontiguousarray(wfc.transpose(0, 2, 1)).astype(bf)
    wprojT = np.ascontiguousarray(wproj.transpose(0, 2, 1)).astype(bf)
    predT = np.ascontiguousarray(pred_w.T)

    cvec = np.zeros((128, 4), np.float32)
    cvec[:, 0] = 1.0 / W
    cvec[:, 1] = 1.0
    cvec[:, 2] = EPS

    if _PROG is None:
        import os
        _PROG = build_program(ln_pb=os.environ.get("LN_PB", "1") == "1",
                              att_pb=os.environ.get("ATT_PB", "1") == "1",
                              att_pipe=os.environ.get("ATT_PIPE", "1") == "1",
                              psum_rec=os.environ.get("PSUM_REC", "0") == "1")
    nc = _PROG

    in_maps = []
    for c in range(8):
        b, slot = c // 4, c % 4
        cf = CORE_FRAMES[slot]
        fr_idx = np.concatenate([np.arange(i * BLK, i * BLK + F) for i in cf])
        tok = np.concatenate([nf_idx, fr_idx])
        h0T = np.ascontiguousarray(fx[b, tok, :].T)
        in_maps.append({
            "h0": h0T,
            "wqkvT": wqkvT, "woT": woT, "wfcT": wfcT, "wprojT": wprojT,
            "predT": predT,
            "nfmask": np.ascontiguousarray(np.repeat(mask[:, cf], F, axis=1)),
            "cvec": cvec,
        })

    res = run_bass_kernel_spmd(nc, in_maps, list(range(8)))

    out = np.zeros((B, N, F, 1024), np.float32)
    for c in range(8):
        b, slot = c // 4, c % 4
        lo = res.results[c]["logits"].reshape(4, F, 1024)
        for si, i in enumerate(CORE_FRAMES[slot]):
            if slot == 3 and si == 0:
                continue
            out[b, i] = lo[si]
    return out.reshape(B, N * F, 1024)
